# revision 1
# baseline (speedup 1.0000x reference)
"""CPFStudent (GNN label propagation + MLP mix) on 8 TRN2 NeuronCores.

Strategy (dst-sharded SpMM with selector matmuls):
  - Reference: 10 PLP steps of plp <- where(mask, hard, A_hat @ plp), with
    A_hat = D^-1/2 (A+I) D^-1/2 built from out-degrees of edge_index[0];
    final logits = sigmoid(alpha)*plp + (1-sigmoid(alpha))*relu(x@W1^T+b1)@W2^T+b2.
  - Only non-train (NT) rows of plp evolve; train (T) rows are constant after
    step 1.  We keep the state as table = dis * plp (dis = deg^-1/2), fp16,
    so per-edge messages need no norm multiply:
        plp_new[d] = dis[d] * ( sum_{e: src NT} table[src] + c )
    where c is a constant per dst: c1 (from dis*label_init over T srcs, used in
    step 1) or c2 (from dis*hard over T srcs, steps 2..10).
  - Nodes are permuted host-side: NT nodes first, padded per-core stripes.
    Each core owns a contiguous stripe of NT dst rows; edges are bucketed by
    (dst_tile of 128, src chunk of <=32768 rows) host-side, padded to uniform
    capacities across cores (SPMD), and gathered per iteration with
    gpsimd.dma_gather (256B elements) from an HBM fp16 table.
  - Scatter/segment-sum is done on the TensorEngine: per 128-edge slot a
    host-precomputed fp8 selector S (S[e,d]=1 iff dst_local(e)==d) multiplies
    the gathered messages, accumulating in PSUM per dst tile.
  - Per-iteration halo exchange: AllGather of each core's new compact fp16
    rows, then a strided DMA expands them into the 256B-strided table.
"""

import math
import os
import sys

import numpy as np

sys.path.insert(0, "/opt/trn_rl_repo")

import ml_dtypes  # noqa: E402

import concourse.bass as bass  # noqa: E402
import concourse.mybir as mybir  # noqa: E402
import concourse.tile as tile  # noqa: E402
from concourse import bacc  # noqa: E402
from concourse.bass_utils import run_bass_kernel_spmd  # noqa: E402

P = 128
NCORES = 8
TPAD = 128  # fp16 elements per table row (256B, dma_gather elem granularity)
GROUP = 7  # dst tiles per dma_gather call group
MAX_CALL = int(os.environ.get("KERNEL_MAX_CALL", "1024"))

F16 = mybir.dt.float16
F32 = mybir.dt.float32
F8 = mybir.dt.float8e4
I16 = mybir.dt.int16
NP_F8 = ml_dtypes.float8_e4m3


def _ceil(a, b):
    return -(-a // b)


class BuildOnly(Exception):
    pass


class EdgePlan:
    """Host-side bucketed edge plan for one SpMM pass, uniform across cores.

    src_row: int array, row index into the pass's gather table
    dst_pid: int array, padded NT id of the destination
    """

    def __init__(self, src_row, dst_pid, n_rows, s_pad, n_tiles):
        self.n_chunks = max(1, _ceil(n_rows, 32768))
        self.chunk = _ceil(n_rows, self.n_chunks)
        self.n_tiles = n_tiles
        nch = self.n_chunks

        core = dst_pid // s_pad
        dloc = dst_pid - core * s_pad
        tl = dloc // P
        dstloc = dloc % P
        ch = src_row // self.chunk

        key = (core * n_tiles + tl) * nch + ch
        counts = np.bincount(key, minlength=NCORES * n_tiles * nch).reshape(
            NCORES, n_tiles, nch
        )
        caps = counts.max(axis=0)  # [n_tiles, nch]
        caps = ((caps + P - 1) // P) * P
        self.caps = caps
        self.slots_per_tile = caps.sum(axis=1) // P  # [n_tiles]
        self.s_off = np.concatenate([[0], np.cumsum(self.slots_per_tile)])
        self.total_slots = int(self.s_off[-1])

        # per (chunk, group) call: num idxs and per-tile column offsets
        self.n_groups = _ceil(n_tiles, GROUP)
        self.call_num = np.zeros((nch, self.n_groups), dtype=np.int64)
        self.buck_col = np.zeros((nch, n_tiles), dtype=np.int64)  # col in its call buf
        for c in range(nch):
            for g in range(self.n_groups):
                off = 0
                for t in range(g * GROUP, min((g + 1) * GROUP, n_tiles)):
                    self.buck_col[c, t] = off
                    off += caps[t, c] // P
                self.call_num[c, g] = off * P
        # col offset of each call inside the flat idx stream (per chunk then group)
        self.call_off = np.zeros((nch, self.n_groups), dtype=np.int64)
        off = 0
        for c in range(nch):
            for g in range(self.n_groups):
                self.call_off[c, g] = off
                off += self.call_num[c, g]
        self.total_idx = off

        # sub-calls of <= MAX_CALL idxs: per (c, g) a list of (idx_off, num, col0)
        self.subcalls = {}
        for c in range(nch):
            for g in range(self.n_groups):
                num = int(self.call_num[c, g])
                base = int(self.call_off[c, g])
                subs = []
                p0 = 0
                while p0 < num:
                    n_ = min(MAX_CALL, num - p0)
                    subs.append((base + p0, n_, p0 // P))
                    p0 += n_
                self.subcalls[(c, g)] = subs

        # order edges by (core, chunk, tile); build padded per-core streams
        order = np.argsort((core * nch + ch) * n_tiles + tl, kind="stable")
        src_o = src_row[order]
        core_o = core[order]
        ch_o = ch[order]
        tl_o = tl[order]
        dst_o = dstloc[order]

        # destination position of each edge in the padded stream
        # padded stream order: for chunk c, group g, tile t in g: cap[t,c] entries
        base_tc = np.zeros((nch, n_tiles), dtype=np.int64)
        for c in range(nch):
            for g in range(self.n_groups):
                for t in range(g * GROUP, min((g + 1) * GROUP, n_tiles)):
                    base_tc[c, t] = self.call_off[c, g] + self.buck_col[c, t] * P

        self.idx16 = np.zeros((NCORES, self.total_idx), dtype=np.int16)
        self.dstloc = np.full((NCORES, self.total_idx), -1, dtype=np.int16)
        # rank of each edge within its (core, chunk, tile) bucket
        grp_key = (core_o * nch + ch_o) * n_tiles + tl_o
        # stable sort keeps original order; compute rank via cumcount
        uniq, inv, cnt = np.unique(grp_key, return_inverse=True, return_counts=True)
        starts = np.concatenate([[0], np.cumsum(cnt)])[:-1]
        rank = np.arange(len(grp_key)) - starts[inv]
        pos = base_tc[ch_o, tl_o] + rank
        self.idx16[core_o, pos] = (src_o - ch_o * self.chunk).astype(np.int16)
        self.dstloc[core_o, pos] = dst_o.astype(np.int16)

    def wrapped_idx(self, core):
        """[128, total_idx//16] int16, wrapped-16 and replicated to 8 groups."""
        v = self.idx16[core].reshape(-1, 16).T  # [16, total/16]
        return np.tile(v, (8, 1)).copy()

    def s_blob(self, core):
        """[128, total_slots*128] fp8: per slot S[e,d] = (dstloc[e]==d).

        Slot order: tile-major (tile t: its chunk-0 slots then chunk-1 slots),
        matching the matmul loop.  Column range of tile t: s_off[t]*128.
        """
        nch = self.n_chunks
        out = np.zeros((P, self.total_slots * P), dtype=NP_F8)
        iota = np.arange(P, dtype=np.int16)
        for t in range(self.n_tiles):
            si = self.s_off[t]
            for c in range(nch):
                nsl = self.caps[t, c] // P
                if nsl == 0:
                    continue
                g = t // GROUP
                base = self.call_off[c, g] + self.buck_col[c, t] * P
                d = self.dstloc[core, base : base + nsl * P].reshape(nsl, P)
                # S [slot, e, d]
                s = (d[:, :, None] == iota[None, None, :]).astype(NP_F8)
                # [P(e), nsl, P(d)] -> columns
                out[:, si * P : (si + nsl) * P] = (
                    s.transpose(1, 0, 2).reshape(P, nsl * P)
                )
                si += nsl
        return out


def _build_program(pm, pc, n_t, s_pad, st_pad, tn, tt):
    """pm: main-pass EdgePlan (NT->NT), pc: c-pass plan (T->NT)."""
    nt_pad = NCORES * s_pad
    nc = bacc.Bacc(None, target_bir_lowering=False, num_devices=NCORES)

    def param(name, shape, dt, out=False):
        return nc.declare_dram_parameter(name, list(shape), dt, isOutput=out)

    tbl_init = param("tbl_init", (nt_pad, TPAD), F16)
    tbl_t1 = param("tbl_t1", (pc.n_chunks * pc.chunk, TPAD), F16)
    tbl_t2 = param("tbl_t2", (pc.n_chunks * pc.chunk, TPAD), F16)
    idx_nt = param("idx_nt", (P, pm.total_idx // 16), I16)
    idx_t = param("idx_t", (P, pc.total_idx // 16), I16)
    s_nt = param("s_nt", (P, pm.total_slots * P), F8)
    s_t = param("s_t", (P, pc.total_slots * P), F8)
    xnt = param("xnt", (512, s_pad), F16)  # pre-transposed on host
    xt = param("xt", (512, st_pad), F16)
    w1t = param("w1t", (512, 256), F16)
    b1 = param("b1", (256, 1), F32)
    w2t = param("w2t", (256, 40), F16)
    b2b = param("b2b", (P, 40), F32)
    alpha_nt = param("alpha_nt", (s_pad, 1), F32)
    alpha_t = param("alpha_t", (st_pad, 1), F32)
    dis_nt = param("dis_nt", (s_pad, 1), F32)
    dissq_nt = param("dissq_nt", (s_pad, 1), F32)
    hard_t = param("hard_t", (st_pad, 40), F32)
    out_nt = param("out_nt", (s_pad, 40), F32, out=True)
    out_t = param("out_t", (st_pad, 40), F32, out=True)

    table = nc.dram_tensor("table", [nt_pad, TPAD], F16)
    cown = nc.dram_tensor("cown", [s_pad, 40], F16)
    callg = nc.dram_tensor("callg", [nt_pad, 40], F16, addr_space="Shared")

    RG = [list(range(NCORES))]

    with tile.TileContext(nc) as tc:
        with (
            tc.tile_pool(name="persist", bufs=1) as pp,
            tc.tile_pool(name="work", bufs=4) as wp,
            tc.tile_pool(name="gpool", bufs=4) as gp,
            tc.tile_pool(name="spool", bufs=3) as sp,
            tc.tile_pool(name="mpsum", bufs=2, space="PSUM") as mp,
            tc.tile_pool(name="apsum", bufs=4, space="PSUM") as ap_,
        ):
            # one-time init: fills pad columns so later strided updates leave
            # only finite data for gathers
            nc.sync.dma_start(out=table[:, :], in_=tbl_init[:, :])

            # ---- persistent SBUF ----
            idxm_sb = pp.tile([P, pm.total_idx // 16], I16, tag="idxm")
            nc.sync.dma_start(out=idxm_sb[:], in_=idx_nt[:, :])
            idxc_sb = pp.tile([P, pc.total_idx // 16], I16, tag="idxc")
            nc.sync.dma_start(out=idxc_sb[:], in_=idx_t[:, :])

            ft_nt = pp.tile([P, tn, 40], F32, tag="ftnt")
            ft_t = pp.tile([P, tt, 40], F32, tag="ftt")
            c1 = pp.tile([P, tn, 40], F32, tag="c1")
            c2 = pp.tile([P, tn, 40], F32, tag="c2")
            compact = pp.tile([P, tn, 40], F16, tag="compact")

            w1_sb = pp.tile([P, 4, 256], F16, tag="w1")
            nc.sync.dma_start(
                out=w1_sb[:], in_=w1t.ap().rearrange("(k p) h -> p k h", p=P)
            )
            w2_sb = pp.tile([P, 2, 40], F16, tag="w2")
            nc.sync.dma_start(
                out=w2_sb[:], in_=w2t.ap().rearrange("(h p) c -> p h c", p=P)
            )
            b1_sb = pp.tile([P, 2], F32, tag="b1")
            nc.sync.dma_start(
                out=b1_sb[:], in_=b1.ap().rearrange("(h p) o -> p (h o)", p=P)
            )
            b2_sb = pp.tile([P, 40], F32, tag="b2")
            nc.sync.dma_start(out=b2_sb[:], in_=b2b[:, :])

            def cols_load(prm, n_tiles, tag):
                t_ = pp.tile([P, n_tiles], F32, tag=tag)
                nc.sync.dma_start(
                    out=t_[:], in_=prm.ap().rearrange("(t p) o -> p (t o)", p=P)
                )
                return t_

            disn_sb = cols_load(dis_nt, tn, "disn")
            dsqn_sb = cols_load(dissq_nt, tn, "dsqn")
            aln_sb = cols_load(alpha_nt, tn, "aln")
            alt_sb = cols_load(alpha_t, tt, "alt")

            # sigmoid(alpha); a*dis; 1-a
            sign_sb = pp.tile([P, tn], F32, tag="sign")
            nc.scalar.activation(
                sign_sb[:], aln_sb[:], mybir.ActivationFunctionType.Sigmoid
            )
            sigt_sb = pp.tile([P, tt], F32, tag="sigt")
            nc.scalar.activation(
                sigt_sb[:], alt_sb[:], mybir.ActivationFunctionType.Sigmoid
            )
            disa_sb = pp.tile([P, tn], F32, tag="disa")
            nc.vector.tensor_tensor(
                out=disa_sb[:], in0=sign_sb[:], in1=disn_sb[:],
                op=mybir.AluOpType.mult,
            )
            oman_sb = pp.tile([P, tn], F32, tag="oman")
            nc.vector.tensor_scalar(
                out=oman_sb[:], in0=sign_sb[:], scalar1=-1.0, scalar2=1.0,
                op0=mybir.AluOpType.mult, op1=mybir.AluOpType.add,
            )
            omat_sb = pp.tile([P, tt], F32, tag="omat")
            nc.vector.tensor_scalar(
                out=omat_sb[:], in0=sigt_sb[:], scalar1=-1.0, scalar2=1.0,
                op0=mybir.AluOpType.mult, op1=mybir.AluOpType.add,
            )

            # ---- MLP (FT branch) ----
            def mlp(xsrc, n_tiles, ft_dst):
                for n in range(n_tiles):
                    xTs = []
                    for k in range(4):
                        xT = wp.tile([P, P], F16, tag="xT")
                        nc.sync.dma_start(
                            out=xT[:],
                            in_=xsrc[k * P : (k + 1) * P, n * P : (n + 1) * P],
                        )
                        xTs.append(xT)
                    ps2 = mp.tile([P, 40], F32, tag="ps2")
                    for h in range(2):
                        ps1 = mp.tile([P, P], F32, tag="ps1")
                        for k in range(4):
                            nc.tensor.matmul(
                                ps1[:],
                                lhsT=w1_sb[:, k, h * P : (h + 1) * P],
                                rhs=xTs[k][:],
                                start=(k == 0),
                                stop=(k == 3),
                            )
                        hT = wp.tile([P, P], F16, tag="hT")
                        nc.scalar.activation(
                            hT[:], ps1[:], mybir.ActivationFunctionType.Relu,
                            bias=b1_sb[:, h : h + 1],
                        )
                        nc.tensor.matmul(
                            ps2[:], lhsT=hT[:], rhs=w2_sb[:, h, :],
                            start=(h == 0), stop=(h == 1),
                        )
                    nc.vector.tensor_tensor(
                        out=ft_dst[:, n, :], in0=ps2[:], in1=b2_sb[:],
                        op=mybir.AluOpType.add,
                    )

            mlp(xnt, tn, ft_nt)
            mlp(xt, tt, ft_t)

            # ---- generic SpMM pass ----
            _regs = {}

            def num_reg(v):
                if v not in _regs:
                    _regs[v] = nc.gpsimd.to_reg(v)
                return _regs[v]

            def spmm_pass(plan, tsrc, idx_sb, s_param, evac):
                """tsrc: DRAM table. evac(t, psum_ap) -> emits eviction."""
                nch = plan.n_chunks
                for g in range(plan.n_groups):
                    gbufs = []
                    for c in range(nch):
                        num = int(plan.call_num[c, g])
                        if num == 0:
                            gbufs.append(None)
                            continue
                        gb = gp.tile([P, num // P, TPAD], F16, tag="gb")
                        r0 = c * plan.chunk
                        nrow = plan.chunk
                        if os.environ.get("KERNEL_NO_GATHER", "0") == "1":
                            # debug: sequential read instead of gather
                            nc.sync.dma_start(
                                out=gb[:],
                                in_=tsrc[r0 : r0 + num, :].rearrange(
                                    "(n p) e -> p n e", p=P
                                ),
                            )
                        else:
                            for off, n_, col0 in plan.subcalls[(c, g)]:
                                nc.gpsimd.dma_gather(
                                    out_ap=gb[:, col0 : col0 + n_ // P, :],
                                    in_ap=tsrc[r0 : r0 + nrow, :],
                                    idxs_ap=idx_sb[:, off // 16 : (off + n_) // 16],
                                    num_idxs=n_,
                                    num_idxs_reg=num_reg(n_),
                                    elem_size=TPAD,
                                )
                        gbufs.append(gb)
                    for t in range(g * GROUP, min((g + 1) * GROUP, plan.n_tiles)):
                        tot = int(plan.slots_per_tile[t])
                        if tot == 0:
                            continue
                        si = int(plan.s_off[t])
                        st_ = sp.tile([P, tot * P], F8, tag="sstr")
                        nc.sync.dma_start(
                            out=st_[:], in_=s_param[:, si * P : (si + tot) * P]
                        )
                        ps = ap_.tile([P, 40], F32, tag="acc")
                        k = 0
                        for c in range(nch):
                            nsl = int(plan.caps[t, c]) // P
                            bc = int(plan.buck_col[c, t])
                            for j in range(nsl):
                                nc.tensor.matmul(
                                    ps[:],
                                    lhsT=st_[:, k * P : (k + 1) * P],
                                    rhs=gbufs[c][:, bc + j, 0:40],
                                    start=(k == 0),
                                    stop=(k == tot - 1),
                                )
                                k += 1
                        evac(t, ps)

            # ---- c1 / c2 passes (T sources; streamed fp8 S) ----
            def evac_c(dst):
                def f(t, ps):
                    nc.vector.tensor_copy(out=dst[:, t, :], in_=ps[:])
                return f

            spmm_pass(pc, tbl_t1, idxc_sb, s_t, evac_c(c1))
            spmm_pass(pc, tbl_t2, idxc_sb, s_t, evac_c(c2))

            # ---- 10 PLP iterations ----
            for it in range(10):
                tsrc = tbl_init if it == 0 else table
                cbuf = c1 if it == 0 else c2

                if it < 9:
                    def evac_iter(t, ps, cbuf=cbuf):
                        tmp = wp.tile([P, 40], F32, tag="ev")
                        nc.vector.tensor_tensor(
                            out=tmp[:], in0=ps[:], in1=cbuf[:, t, :],
                            op=mybir.AluOpType.add,
                        )
                        nc.vector.tensor_scalar(
                            out=compact[:, t, :], in0=tmp[:],
                            scalar1=dsqn_sb[:, t : t + 1], scalar2=None,
                            op0=mybir.AluOpType.mult,
                        )
                else:
                    def evac_iter(t, ps, cbuf=cbuf):
                        tmp = wp.tile([P, 40], F32, tag="ev")
                        nc.vector.tensor_tensor(
                            out=tmp[:], in0=ps[:], in1=cbuf[:, t, :],
                            op=mybir.AluOpType.add,
                        )
                        t2 = wp.tile([P, 40], F32, tag="ev2")
                        nc.vector.tensor_scalar(
                            out=t2[:], in0=tmp[:],
                            scalar1=disa_sb[:, t : t + 1], scalar2=None,
                            op0=mybir.AluOpType.mult,
                        )
                        t3 = wp.tile([P, 40], F32, tag="ev3")
                        nc.vector.tensor_scalar(
                            out=t3[:], in0=ft_nt[:, t, :],
                            scalar1=oman_sb[:, t : t + 1], scalar2=None,
                            op0=mybir.AluOpType.mult,
                        )
                        t4 = wp.tile([P, 40], F32, tag="ev4")
                        nc.vector.tensor_tensor(
                            out=t4[:], in0=t2[:], in1=t3[:],
                            op=mybir.AluOpType.add,
                        )
                        nc.sync.dma_start(
                            out=out_nt[t * P : (t + 1) * P, :], in_=t4[:]
                        )

                spmm_pass(pm, tsrc, idxm_sb, s_nt, evac_iter)

                if it < 9:
                    nc.sync.dma_start(
                        out=cown.ap().rearrange("(t p) c -> p t c", p=P),
                        in_=compact[:],
                    )
                    if os.environ.get("KERNEL_NO_CC", "0") == "1":
                        # debug mode: skip the collective (wrong cross-core data)
                        nc.sync.dma_start(
                            out=callg[0 : s_pad, :], in_=cown[:, :]
                        )
                    else:
                        nc.gpsimd.collective_compute(
                            "AllGather",
                            mybir.AluOpType.bypass,
                            replica_groups=RG,
                            ins=[cown.ap().opt()],
                            outs=[callg.ap().opt()],
                        )
                    nc.sync.dma_start(out=table[:, 0:40], in_=callg[:, :])

            # ---- T-side final combine ----
            for t in range(tt):
                hsb = wp.tile([P, 40], F32, tag="hsb")
                nc.sync.dma_start(out=hsb[:], in_=hard_t[t * P : (t + 1) * P, :])
                t1_ = wp.tile([P, 40], F32, tag="tc1")
                nc.vector.tensor_scalar(
                    out=t1_[:], in0=hsb[:], scalar1=sigt_sb[:, t : t + 1],
                    scalar2=None, op0=mybir.AluOpType.mult,
                )
                t2_ = wp.tile([P, 40], F32, tag="tc2")
                nc.vector.tensor_scalar(
                    out=t2_[:], in0=ft_t[:, t, :], scalar1=omat_sb[:, t : t + 1],
                    scalar2=None, op0=mybir.AluOpType.mult,
                )
                t3_ = wp.tile([P, 40], F32, tag="tc3")
                nc.vector.tensor_tensor(
                    out=t3_[:], in0=t1_[:], in1=t2_[:],
                    op=mybir.AluOpType.add,
                )
                nc.sync.dma_start(out=out_t[t * P : (t + 1) * P, :], in_=t3_[:])

    nc.compile()
    return nc


def kernel(**inputs):
    x = np.asarray(inputs["x"], dtype=np.float32)
    edge_index = np.asarray(inputs["edge_index"])
    label_init = np.asarray(inputs["label_init"], dtype=np.float32)
    train_mask = np.asarray(inputs["train_mask"]).astype(bool)
    hard = np.asarray(inputs["hard_one_hot"], dtype=np.float32)
    fc1_w = np.asarray(inputs["fc1_w"], dtype=np.float32)
    fc1_b = np.asarray(inputs["fc1_b"], dtype=np.float32)
    fc2_w = np.asarray(inputs["fc2_w"], dtype=np.float32)
    fc2_b = np.asarray(inputs["fc2_b"], dtype=np.float32)
    alpha = np.asarray(inputs["alpha"], dtype=np.float32)

    n = x.shape[0]
    row = edge_index[0].astype(np.int64)
    col = edge_index[1].astype(np.int64)

    deg = np.bincount(row, minlength=n).astype(np.float64) + 1.0
    dis = (1.0 / np.sqrt(deg)).astype(np.float32)

    nt_ids = np.nonzero(~train_mask)[0]
    t_ids = np.nonzero(train_mask)[0]
    n_nt, n_t = len(nt_ids), len(t_ids)

    s_real = _ceil(n_nt, NCORES)
    tn = _ceil(s_real, P)
    s_pad = tn * P
    nt_pad = NCORES * s_pad
    st_real = _ceil(n_t, NCORES)
    tt = _ceil(st_real, P)
    st_pad = tt * P

    # padded NT id / compact T id for each original node
    pid = np.full(n, -1, dtype=np.int64)
    j = np.arange(n_nt)
    stripe = j // s_real
    pid[nt_ids] = stripe * s_pad + (j - stripe * s_real)
    tix = np.full(n, -1, dtype=np.int64)
    tix[t_ids] = np.arange(n_t)

    # edges into NT dsts
    sel = ~train_mask[col]
    es, ed = row[sel], col[sel]
    src_nt = ~train_mask[es]
    # main: NT->NT plus self-loops on NT
    m_src = np.concatenate([pid[es[src_nt]], pid[nt_ids]])
    m_dst = np.concatenate([pid[ed[src_nt]], pid[nt_ids]])
    pm = EdgePlan(m_src, m_dst, nt_pad, s_pad, tn)
    # cpass: T->NT
    c_src = tix[es[~src_nt]]
    c_dst = pid[ed[~src_nt]]
    pc = EdgePlan(c_src, c_dst, n_t, s_pad, tn)

    # ---- tables ----
    scaled_li = dis[:, None] * label_init  # [n, 40]
    scaled_hd = dis[:, None] * hard

    def pack_rows(rows40):
        out = np.zeros((rows40.shape[0], TPAD), dtype=np.float16)
        out[:, :40] = rows40.astype(np.float16)
        return out

    tbl_init_g = np.zeros((nt_pad, TPAD), dtype=np.float16)
    tbl_init_g[pid[nt_ids], :40] = scaled_li[nt_ids].astype(np.float16)
    t_rows = pc.n_chunks * pc.chunk
    tbl_t1_g = np.zeros((t_rows, TPAD), dtype=np.float16)
    tbl_t1_g[: n_t, :40] = scaled_li[t_ids].astype(np.float16)
    tbl_t2_g = np.zeros((t_rows, TPAD), dtype=np.float16)
    tbl_t2_g[: n_t, :40] = scaled_hd[t_ids].astype(np.float16)

    # ---- per-core MLP / combine inputs ----
    def stripe_rows(ids, srl, spad_, nstripes=NCORES):
        """Return [nstripes, spad_] original-id per padded slot (-1 pad)."""
        m = np.full((nstripes, spad_), -1, dtype=np.int64)
        for i in range(nstripes):
            lo = i * srl
            hi = min(len(ids), (i + 1) * srl)
            if hi > lo:
                m[i, : hi - lo] = ids[lo:hi]
        return m

    nt_map = stripe_rows(nt_ids, s_real, s_pad)
    t_map = stripe_rows(t_ids, st_real, st_pad)

    def take(arr, idmap, fill=0.0):
        out = np.full((idmap.shape[0], idmap.shape[1]) + arr.shape[1:], fill,
                      dtype=arr.dtype)
        valid = idmap >= 0
        out[valid] = arr[idmap[valid]]
        return out

    xnt_g = np.ascontiguousarray(
        take(x, nt_map).astype(np.float16).transpose(0, 2, 1)
    )
    xt_g = np.ascontiguousarray(take(x, t_map).astype(np.float16).transpose(0, 2, 1))
    al_nt_g = take(alpha, nt_map).astype(np.float32)
    al_t_g = take(alpha, t_map).astype(np.float32)
    dis_nt_g = take(dis[:, None], nt_map).astype(np.float32)
    dsq_nt_g = take((dis * dis)[:, None], nt_map).astype(np.float32)
    hard_t_g = take(hard, t_map).astype(np.float32)

    w1t_g = fc1_w.T.astype(np.float16).copy()  # [512, 256]
    b1_g = fc1_b.reshape(256, 1).astype(np.float32)
    w2t_g = fc2_w.T.astype(np.float16).copy()  # [256, 40]
    b2b_g = np.tile(fc2_b.reshape(1, 40), (P, 1)).astype(np.float32)

    nc = _build_program(pm, pc, n_t, s_pad, st_pad, tn, tt)

    if os.environ.get("KERNEL_BUILD_ONLY", "0") == "1":
        e = BuildOnly()
        e.nc = nc
        raise e

    in_maps = []
    for i in range(NCORES):
        in_maps.append(
            dict(
                tbl_init=tbl_init_g,
                tbl_t1=tbl_t1_g,
                tbl_t2=tbl_t2_g,
                idx_nt=pm.wrapped_idx(i),
                idx_t=pc.wrapped_idx(i),
                s_nt=pm.s_blob(i),
                s_t=pc.s_blob(i),
                xnt=xnt_g[i],
                xt=xt_g[i],
                w1t=w1t_g,
                b1=b1_g,
                w2t=w2t_g,
                b2b=b2b_g,
                alpha_nt=al_nt_g[i],
                alpha_t=al_t_g[i],
                dis_nt=dis_nt_g[i],
                dissq_nt=dsq_nt_g[i],
                hard_t=hard_t_g[i],
            )
        )

    if os.environ.get("KERNEL_SIM", "0") == "1":
        from concourse import bass_interp

        sim = bass_interp.MultiCoreSim(nc, NCORES)
        for i in range(NCORES):
            for k, v in in_maps[i].items():
                sim.cores[i].tensor(k)[:] = v
        sim.simulate()
        results = [
            {k: np.array(sim.cores[i].mem_tensor(k)) for k in ("out_nt", "out_t")}
            for i in range(NCORES)
        ]
        res = None
    else:
        res = run_bass_kernel_spmd(
            nc, in_maps, core_ids=list(range(NCORES)),
            trace=bool(int(os.environ.get("KERNEL_TRACE", "0"))),
        )
        results = res.results
        nbench = int(os.environ.get("KERNEL_BENCH", "0"))
        if nbench > 0:
            import time as _time

            times = []
            for _ in range(nbench):
                t0 = _time.time()
                run_bass_kernel_spmd(nc, in_maps, core_ids=list(range(NCORES)))
                times.append(_time.time() - t0)
            kernel.last_bench_s = min(times)
    kernel.last_results = res
    kernel.last_nc = nc
    kernel.last_in_maps = in_maps

    out = np.zeros((n, 40), dtype=np.float32)
    for i in range(NCORES):
        om = results[i]["out_nt"]
        ot = results[i]["out_t"]
        v = nt_map[i] >= 0
        out[nt_map[i][v]] = om[v]
        v = t_map[i] >= 0
        out[t_map[i][v]] = ot[v]
    return out



# revision 21
# speedup vs baseline: 2.6121x; 2.6121x over previous
"""CPFStudent (GNN label propagation + MLP mix) on 8 TRN2 NeuronCores.

Strategy v2 (column-gather + vector-engine segment reduce):
  - Reference: 10 PLP steps of plp <- where(mask, hard, A_hat @ plp), with
    A_hat = D^-1/2 (A+I) D^-1/2 built from out-degrees of edge_index[0];
    final logits = sigmoid(alpha)*plp + (1-sigmoid(alpha))*relu(x@W1^T+b1)@W2^T+b2.
  - Only non-train (NT) rows of plp evolve.  State kept as table = dis * plp
    (dis = deg^-1/2), fp16, 256B-strided rows in HBM:
        plp_new[d] = dis[d] * ( sum_{e: src NT} table[src] + c[d] )
    c from T sources is constant (c1 for step 1 from label_init, c2 after).
  - NT nodes are sorted by in-degree and dealt into 128-node tiles; tiles are
    assigned to cores round-robin so per-core load balances and tiles are
    degree-homogeneous.
  - Per dst tile, in-edges are laid out COLUMN-major: gather writes message
    j of dst d to [partition d, column j]; short dsts padded with a known
    zero table row.  Segment sum = ONE strided tensor_reduce per tile on the
    vector engine (no selector matmuls, no S blobs) — this kernel is
    instruction-dispatch-bound on real HW, so instruction count is king.
  - Sources split in 2 chunks so gather indices fit int16.
  - Per-iteration halo exchange: AllGather of compact fp16 rows, then a
    strided DMA expands them into the 256B-strided table.
"""

import math
import os
import sys

import numpy as np

sys.path.insert(0, "/opt/trn_rl_repo")

import concourse.bass as bass  # noqa: E402
import concourse.mybir as mybir  # noqa: E402
import concourse.tile as tile  # noqa: E402
from concourse import bacc  # noqa: E402
from concourse.bass_utils import run_bass_kernel_spmd  # noqa: E402

P = 128
NCORES = 8
TPAD = 128  # fp16 elements per table row (256B, dma_gather elem granularity)
MAX_CALL = int(os.environ.get("KERNEL_MAX_CALL", "2048"))
SCRATCH = int(os.environ.get("KERNEL_SCRATCH", "65536"))
NQUEUES = int(os.environ.get("KERNEL_QUEUES", "1"))

F16 = mybir.dt.float16
F32 = mybir.dt.float32
I16 = mybir.dt.int16


def _ceil(a, b):
    return -(-a // b)


class BuildOnly(Exception):
    pass


class ColPlan:
    """Column-major per-dst edge layout for one gather+reduce pass.

    src_row: global row index into the pass's source table
    dst_pid: padded NT id of the destination
    Layout (identical across cores): tile-major; per tile, chunk-0 columns
    then chunk-1 columns; within a (tile, chunk) block column j of dst slot p
    sits at flat position block_off + j*128 + p.  Unused slots hold the
    chunk's known-zero row.
    """

    def __init__(self, src_row, dst_pid, chunk, n_chunks, zero_rel, s_pad, tn):
        self.chunk = chunk
        self.n_chunks = n_chunks
        self.tn = tn

        core = dst_pid // s_pad
        dloc = dst_pid % s_pad
        tl = dloc // P
        slot = dloc % P
        ch = src_row // chunk
        rel = src_row - ch * chunk

        key = ((core * tn + tl) * n_chunks + ch) * P + slot
        counts = np.bincount(
            key, minlength=NCORES * tn * n_chunks * P
        ).reshape(NCORES, tn, n_chunks, P)
        caps = counts.max(axis=(0, 3))  # [tn, n_chunks]
        self.caps = caps
        self.cols_per_tile = caps.sum(axis=1)  # [tn]

        blk = caps * P
        flat_off = np.zeros((tn, n_chunks), dtype=np.int64)
        off = 0
        for t in range(tn):
            for c in range(n_chunks):
                flat_off[t, c] = off
                off += int(blk[t, c])
        self.total_idx = off
        # idx-stream offset of each tile's first block (+ total sentinel)
        self.tile_off = np.concatenate([flat_off[:, 0], [off]])

        idx = np.empty((NCORES, off), dtype=np.int16)
        for t in range(tn):
            for c in range(n_chunks):
                idx[:, flat_off[t, c] : flat_off[t, c] + blk[t, c]] = zero_rel[c]

        # rank of each edge within its (core, tile, chunk, slot) bucket
        order = np.argsort(key, kind="stable")
        key_o = key[order]
        uniq, inv, cnt = np.unique(key_o, return_inverse=True, return_counts=True)
        starts = np.concatenate([[0], np.cumsum(cnt)])[:-1]
        rank_o = np.arange(len(key_o)) - starts[inv]
        rank = np.empty_like(rank_o)
        rank[order] = rank_o

        pos = flat_off[tl, ch] + rank * P + slot
        idx[core, pos] = rel.astype(np.int16)
        self.idx16 = idx

        # calls: per (tile, chunk) block, subcalls of <= MAX_CALL idxs
        self.calls = {t: [] for t in range(tn)}  # (chunk, idx_off, n, col0)
        self.n_calls = 0
        for t in range(tn):
            col0 = 0
            for c in range(n_chunks):
                n = int(blk[t, c])
                base = int(flat_off[t, c])
                p0 = 0
                while p0 < n:
                    k = min(MAX_CALL, n - p0)
                    self.calls[t].append((c, base + p0, k, col0 + p0 // P))
                    self.n_calls += 1
                    p0 += k
                col0 += int(caps[t, c])

    def wrapped_idx(self, core):
        """[128, total_idx//16] int16, wrapped-16 and replicated to 8 groups."""
        v = self.idx16[core].reshape(-1, 16).T  # [16, total/16]
        return np.tile(v, (8, 1)).copy()


def _build_program(pm, pc, s_pad, st_pad, tn, tt, tbl_rows, tblt_rows):
    nt_pad = NCORES * s_pad
    nc = bacc.Bacc(
        None,
        target_bir_lowering=False,
        num_devices=NCORES,
        dynamic_dma_scratch_size=SCRATCH,
        num_swdge_queues=NQUEUES,
    )

    def param(name, shape, dt, out=False):
        return nc.declare_dram_parameter(name, list(shape), dt, isOutput=out)

    tbl_init = param("tbl_init", (tbl_rows, TPAD), F16)
    tbl_t1 = param("tbl_t1", (tblt_rows, TPAD), F16)
    tbl_t2 = param("tbl_t2", (tblt_rows, TPAD), F16)
    idx_nt = param("idx_nt", (P, pm.total_idx // 16), I16)
    idx_t = param("idx_t", (P, pc.total_idx // 16), I16)
    xnt = param("xnt", (512, s_pad), F16)  # pre-transposed on host
    xt = param("xt", (512, st_pad), F16)
    w1t = param("w1t", (512, 256), F16)
    b1 = param("b1", (256, 1), F32)
    w2t = param("w2t", (256, 40), F16)
    b2b = param("b2b", (P, 40), F32)
    alpha_nt = param("alpha_nt", (s_pad, 1), F32)
    alpha_t = param("alpha_t", (st_pad, 1), F32)
    dis_nt = param("dis_nt", (s_pad, 1), F32)
    dissq_nt = param("dissq_nt", (s_pad, 1), F32)
    hard_t = param("hard_t", (st_pad, 40), F32)
    out_nt = param("out_nt", (s_pad, 40), F32, out=True)
    out_t = param("out_t", (st_pad, 40), F32, out=True)

    table = nc.dram_tensor("table", [tbl_rows, TPAD], F16)
    # ping-pong the collective in/out buffers: a lagging peer may still be
    # pulling iteration k's data after our collective instruction completed,
    # so iteration k+1 must not overwrite the same buffers
    cown = [nc.dram_tensor(f"cown{i}", [s_pad, 40], F16) for i in range(2)]
    callg = [
        nc.dram_tensor(f"callg{i}", [nt_pad, 40], F16, addr_space="Shared")
        for i in range(2)
    ]

    RG = [list(range(NCORES))]
    CH_M = pm.chunk
    CH_T = pc.chunk

    with tile.TileContext(nc) as tc:
        with (
            tc.tile_pool(name="persist", bufs=1) as pp,
            tc.tile_pool(name="work", bufs=4) as wp,
            tc.tile_pool(name="bigtmp", bufs=1) as bp,
            tc.tile_pool(name="gpool", bufs=4) as gp,
            tc.tile_pool(name="mpsum", bufs=2, space="PSUM") as mp,
            tc.tile_pool(name="mpsum2", bufs=4, space="PSUM") as mp2,
        ):
            # one-time init: fills pad columns so later strided updates leave
            # only finite data for gathers
            nc.sync.dma_start(out=table[:, :], in_=tbl_init[:, :])

            # ---- persistent SBUF ----
            idxm_sb = pp.tile([P, pm.total_idx // 16], I16, tag="idxm")
            nc.sync.dma_start(out=idxm_sb[:], in_=idx_nt[:, :])

            ft_nt = pp.tile([P, tn, 40], F32, tag="ftnt")
            ft_t = pp.tile([P, tt, 40], F32, tag="ftt")
            c1 = pp.tile([P, tn, 40], F32, tag="c1")
            c2 = pp.tile([P, tn, 40], F32, tag="c2")
            sums = pp.tile([P, tn, 40], F32, tag="sums")
            compact = pp.tile([P, tn, 40], F16, tag="compact")
            # pad tiles are skipped by every pass but their rows flow to the
            # table via cown/callg each iteration — they must stay zero (they
            # are the gather-padding zero rows), and the batched evac reads
            # whole buffers, so zero everything once
            nc.vector.memset(compact[:], 0.0)
            nc.vector.memset(sums[:], 0.0)
            nc.vector.memset(c1[:], 0.0)
            nc.vector.memset(c2[:], 0.0)

            def bc40(ap):
                """[P, n] -> [P, n, 40] stride-0 broadcast view."""
                return bass.AP(ap.tensor, ap.offset, [*ap.ap, [0, 40]])

            w1_sb = pp.tile([P, 4, 256], F16, tag="w1")
            nc.sync.dma_start(
                out=w1_sb[:], in_=w1t.ap().rearrange("(k p) h -> p k h", p=P)
            )
            w2_sb = pp.tile([P, 2, 40], F16, tag="w2")
            nc.sync.dma_start(
                out=w2_sb[:], in_=w2t.ap().rearrange("(h p) c -> p h c", p=P)
            )
            b1_sb = pp.tile([P, 2], F32, tag="b1")
            nc.sync.dma_start(
                out=b1_sb[:], in_=b1.ap().rearrange("(h p) o -> p (h o)", p=P)
            )
            b2_sb = pp.tile([P, 40], F32, tag="b2")
            nc.sync.dma_start(out=b2_sb[:], in_=b2b[:, :])

            def cols_load(prm, n_tiles, tag):
                t_ = pp.tile([P, n_tiles], F32, tag=tag)
                nc.sync.dma_start(
                    out=t_[:], in_=prm.ap().rearrange("(t p) o -> p (t o)", p=P)
                )
                return t_

            disn_sb = cols_load(dis_nt, tn, "disn")
            dsqn_sb = cols_load(dissq_nt, tn, "dsqn")
            aln_sb = cols_load(alpha_nt, tn, "aln")
            alt_sb = cols_load(alpha_t, tt, "alt")

            # sigmoid(alpha); a*dis; 1-a
            sign_sb = pp.tile([P, tn], F32, tag="sign")
            nc.scalar.activation(
                sign_sb[:], aln_sb[:], mybir.ActivationFunctionType.Sigmoid
            )
            sigt_sb = pp.tile([P, tt], F32, tag="sigt")
            nc.scalar.activation(
                sigt_sb[:], alt_sb[:], mybir.ActivationFunctionType.Sigmoid
            )
            disa_sb = pp.tile([P, tn], F32, tag="disa")
            nc.vector.tensor_tensor(
                out=disa_sb[:], in0=sign_sb[:], in1=disn_sb[:],
                op=mybir.AluOpType.mult,
            )
            oman_sb = pp.tile([P, tn], F32, tag="oman")
            nc.vector.tensor_scalar(
                out=oman_sb[:], in0=sign_sb[:], scalar1=-1.0, scalar2=1.0,
                op0=mybir.AluOpType.mult, op1=mybir.AluOpType.add,
            )
            omat_sb = pp.tile([P, tt], F32, tag="omat")
            nc.vector.tensor_scalar(
                out=omat_sb[:], in0=sigt_sb[:], scalar1=-1.0, scalar2=1.0,
                op0=mybir.AluOpType.mult, op1=mybir.AluOpType.add,
            )

            # ---- MLP (FT branch), 4 node-tiles per matmul group ----
            def mlp(xsrc, n_tiles, ft_dst):
                g0 = 0
                while g0 < n_tiles:
                    gw = min(4, n_tiles - g0)
                    W = gw * P
                    xT4s = []
                    for k in range(4):
                        xT4 = wp.tile([P, W], F16, tag="xT4")
                        nc.sync.dma_start(
                            out=xT4[:],
                            in_=xsrc[k * P : (k + 1) * P, g0 * P : g0 * P + W],
                        )
                        xT4s.append(xT4)
                    ps2s = [
                        mp2.tile([P, 40], F32, tag="ps2", name=f"ps2_{j}")
                        for j in range(gw)
                    ]
                    for h in range(2):
                        ps1 = mp.tile([P, W], F32, tag="ps1")
                        for k in range(4):
                            nc.tensor.matmul(
                                ps1[:],
                                lhsT=w1_sb[:, k, h * P : (h + 1) * P],
                                rhs=xT4s[k][:],
                                start=(k == 0),
                                stop=(k == 3),
                            )
                        hT = wp.tile([P, W], F16, tag="hT4")
                        nc.scalar.activation(
                            hT[:], ps1[:], mybir.ActivationFunctionType.Relu,
                            bias=b1_sb[:, h : h + 1],
                        )
                        for j in range(gw):
                            nc.tensor.matmul(
                                ps2s[j][:],
                                lhsT=hT[:, j * P : (j + 1) * P],
                                rhs=w2_sb[:, h, :],
                                start=(h == 0), stop=(h == 1),
                            )
                    for j in range(gw):
                        nc.vector.tensor_tensor(
                            out=ft_dst[:, g0 + j, :], in0=ps2s[j][:],
                            in1=b2_sb[:], op=mybir.AluOpType.add,
                        )
                    g0 += gw

            mlp(xnt, tn, ft_nt)
            mlp(xt, tt, ft_t)

            # ---- generic column-gather + reduce pass ----
            _regs = {}

            def num_reg(v):
                if v not in _regs:
                    _regs[v] = nc.gpsimd.to_reg(v)
                return _regs[v]

            def col_pass(plan, tsrc, idx_sb, ch_rows, dst,
                         t_lo=0, t_hi=None, idx_base=0):
                """Gathers per dst tile + one strided reduce into dst[:, t, :].

                t_lo/t_hi restrict to a tile range; idx_base is the stream
                offset of idx_sb[0] within the plan's flat idx stream."""
                qi = 0
                if t_hi is None:
                    t_hi = plan.tn
                for t in range(t_lo, t_hi):
                    cols = int(plan.cols_per_tile[t])
                    if cols == 0:
                        continue
                    gb = gp.tile([P, cols, TPAD], F16, tag="gb")
                    for c, off, n_, col0 in plan.calls[t]:
                        off -= idx_base
                        nc.gpsimd.dma_gather(
                            out_ap=gb[:, col0 : col0 + n_ // P, :],
                            in_ap=tsrc[c * ch_rows : (c + 1) * ch_rows, :],
                            idxs_ap=idx_sb[:, off // 16 : (off + n_) // 16],
                            num_idxs=n_,
                            num_idxs_reg=num_reg(n_),
                            elem_size=TPAD,
                            queue_num=qi % NQUEUES,
                        )
                        qi += 1
                    nc.vector.tensor_reduce(
                        out=dst[:, t, :],
                        in_=gb[:, :, 0:40].rearrange("p n e -> p e n"),
                        axis=mybir.AxisListType.X,
                        op=mybir.AluOpType.add,
                    )

            # ---- c1 / c2 passes (T sources) ----
            # split tiles at ~half the idx stream so the buffer stays small
            t_mid = pc.tn // 2
            while pc.tile_off[t_mid] % 16 != 0:
                t_mid += 1
            splits = [(0, t_mid), (t_mid, pc.tn)]
            buf_cols = max(
                (pc.tile_off[hi] - pc.tile_off[lo]) // 16 for lo, hi in splits
            )
            with tc.tile_pool(name="cidx", bufs=1) as cp:
                for lo, hi in splits:
                    base = int(pc.tile_off[lo])
                    ncols = (int(pc.tile_off[hi]) - base) // 16
                    if ncols == 0:
                        continue
                    idxc_sb = cp.tile([P, buf_cols], I16, tag="idxc")
                    nc.sync.dma_start(
                        out=idxc_sb[:, 0:ncols],
                        in_=idx_t[:, base // 16 : base // 16 + ncols],
                    )
                    col_pass(pc, tbl_t1, idxc_sb, CH_T, c1, lo, hi, base)
                    col_pass(pc, tbl_t2, idxc_sb, CH_T, c2, lo, hi, base)

            # ---- 10 PLP iterations ----
            for it in range(10):
                tsrc = tbl_init if it == 0 else table
                cbuf = c1 if it == 0 else c2

                col_pass(pm, tsrc, idxm_sb, CH_M, sums)

                if it < 9:
                    # compact = dsq * (sums + c); batched over all tiles
                    tmp = bp.tile([P, tn, 40], F32, tag="ev")
                    nc.vector.tensor_tensor(
                        out=tmp[:], in0=sums[:], in1=cbuf[:],
                        op=mybir.AluOpType.add,
                    )
                    nc.vector.tensor_tensor(
                        out=compact[:], in0=tmp[:], in1=bc40(dsqn_sb[:]),
                        op=mybir.AluOpType.mult,
                    )
                else:
                    # out = a*dis*(sums + c) + (1-a)*ft; batched, in place
                    tmp = bp.tile([P, tn, 40], F32, tag="ev")
                    nc.vector.tensor_tensor(
                        out=tmp[:], in0=sums[:], in1=cbuf[:],
                        op=mybir.AluOpType.add,
                    )
                    nc.vector.tensor_tensor(
                        out=tmp[:], in0=tmp[:], in1=bc40(disa_sb[:]),
                        op=mybir.AluOpType.mult,
                    )
                    t3 = bp.tile([P, tn, 40], F32, tag="ev3")
                    nc.vector.tensor_tensor(
                        out=t3[:], in0=ft_nt[:], in1=bc40(oman_sb[:]),
                        op=mybir.AluOpType.mult,
                    )
                    nc.vector.tensor_tensor(
                        out=tmp[:], in0=tmp[:], in1=t3[:],
                        op=mybir.AluOpType.add,
                    )
                    nc.sync.dma_start(
                        out=out_nt.ap().rearrange("(t p) c -> p t c", p=P),
                        in_=tmp[:],
                    )

                if it < 9:
                    cw, cg = cown[it % 2], callg[it % 2]
                    nc.sync.dma_start(
                        out=cw.ap().rearrange("(t p) c -> p t c", p=P),
                        in_=compact[:],
                    )
                    if os.environ.get("KERNEL_NO_CC", "0") == "1":
                        # debug mode: skip the collective (wrong cross-core data)
                        nc.sync.dma_start(
                            out=cg[0 : s_pad, :], in_=cw[:, :]
                        )
                    else:
                        nc.gpsimd.collective_compute(
                            "AllGather",
                            mybir.AluOpType.bypass,
                            replica_groups=RG,
                            ins=[cw.ap().opt()],
                            outs=[cg.ap().opt()],
                        )
                    nc.sync.dma_start(out=table[0:nt_pad, 0:40], in_=cg[:, :])

            # ---- T-side final combine (batched) ----
            hsb = bp.tile([P, tt, 40], F32, tag="ev")
            nc.sync.dma_start(
                out=hsb[:], in_=hard_t.ap().rearrange("(t p) c -> p t c", p=P)
            )
            nc.vector.tensor_tensor(
                out=hsb[:], in0=hsb[:], in1=bc40(sigt_sb[:]),
                op=mybir.AluOpType.mult,
            )
            t2_ = bp.tile([P, tt, 40], F32, tag="ev3")
            nc.vector.tensor_tensor(
                out=t2_[:], in0=ft_t[:], in1=bc40(omat_sb[:]),
                op=mybir.AluOpType.mult,
            )
            nc.vector.tensor_tensor(
                out=hsb[:], in0=hsb[:], in1=t2_[:],
                op=mybir.AluOpType.add,
            )
            nc.sync.dma_start(
                out=out_t.ap().rearrange("(t p) c -> p t c", p=P), in_=hsb[:]
            )

    nc.compile()
    return nc


def kernel(**inputs):
    x = np.asarray(inputs["x"], dtype=np.float32)
    edge_index = np.asarray(inputs["edge_index"])
    label_init = np.asarray(inputs["label_init"], dtype=np.float32)
    train_mask = np.asarray(inputs["train_mask"]).astype(bool)
    hard = np.asarray(inputs["hard_one_hot"], dtype=np.float32)
    fc1_w = np.asarray(inputs["fc1_w"], dtype=np.float32)
    fc1_b = np.asarray(inputs["fc1_b"], dtype=np.float32)
    fc2_w = np.asarray(inputs["fc2_w"], dtype=np.float32)
    fc2_b = np.asarray(inputs["fc2_b"], dtype=np.float32)
    alpha = np.asarray(inputs["alpha"], dtype=np.float32)

    n = x.shape[0]
    row = edge_index[0].astype(np.int64)
    col = edge_index[1].astype(np.int64)

    deg = np.bincount(row, minlength=n).astype(np.float64) + 1.0
    dis = (1.0 / np.sqrt(deg)).astype(np.float32)

    nt_ids = np.nonzero(~train_mask)[0]
    t_ids = np.nonzero(train_mask)[0]
    n_nt, n_t = len(nt_ids), len(t_ids)

    # tiles of 128 NT nodes, degree-sorted, dealt round-robin to cores.
    # +1 tile rank per core so every core ends with an all-pad (zero) tile,
    # giving each chunk a known zero row for gather padding.
    n_tiles_real = _ceil(n_nt, P)
    tn = _ceil(n_tiles_real, NCORES) + 1
    s_pad = tn * P
    nt_pad = NCORES * s_pad
    assert nt_pad % 2 == 0
    CH_M = nt_pad // 2  # main-table chunk rows

    st_real = _ceil(n_t, NCORES)
    tt = _ceil(st_real, P)
    st_pad = tt * P

    # in-degree of each node counting NT sources + self loop (what the main
    # pass pads against)
    sel_nt_src = ~train_mask[row]
    deg_in_nt = np.bincount(col[sel_nt_src], minlength=n) + 1

    order = np.argsort(-deg_in_nt[nt_ids], kind="stable")  # NT rank by degree
    ranked = nt_ids[order]  # i-th highest-degree NT node
    i_arr = np.arange(n_nt)
    k_tile = i_arr // P
    core_of = k_tile % NCORES
    tin_of = k_tile // NCORES
    pid = np.full(n, -1, dtype=np.int64)
    pid[ranked] = core_of * s_pad + tin_of * P + (i_arr % P)

    # main zero row: every core's last tile is all-pad; cores 0 and NCORES//2
    # start chunks 0/1, so relative offset (tn-1)*P is zero in both chunks
    zero_m = (tn - 1) * P
    assert zero_m < CH_M

    # T table: row 0 is a zero row, rows 1..n_t real; 2 chunks; chunk 1 must
    # end with at least one zero (pad) row
    tblt_rows_min = n_t + 1
    CH_T = _ceil(tblt_rows_min, 2)
    if 2 * CH_T == tblt_rows_min:
        CH_T += 64
    tblt_rows = 2 * CH_T
    assert tblt_rows > tblt_rows_min
    zero_t = (0, max(tblt_rows_min - CH_T, 0))  # per-chunk zero row (relative)
    assert 0 <= zero_t[1] < CH_T
    tix = np.full(n, -1, dtype=np.int64)
    tix[t_ids] = 1 + np.arange(n_t)

    # edges into NT dsts
    sel = ~train_mask[col]
    es, ed = row[sel], col[sel]
    src_nt = ~train_mask[es]
    # main: NT->NT plus self-loops on NT
    m_src = np.concatenate([pid[es[src_nt]], pid[nt_ids]])
    m_dst = np.concatenate([pid[ed[src_nt]], pid[nt_ids]])
    pm = ColPlan(m_src, m_dst, CH_M, 2, (zero_m, zero_m), s_pad, tn)
    # cpass: T->NT
    c_src = tix[es[~src_nt]]
    c_dst = pid[ed[~src_nt]]
    pc = ColPlan(c_src, c_dst, CH_T, 2, zero_t, s_pad, tn)

    # ---- tables ----
    scaled_li = dis[:, None] * label_init  # [n, 40]
    scaled_hd = dis[:, None] * hard

    tbl_init_g = np.zeros((nt_pad, TPAD), dtype=np.float16)
    tbl_init_g[pid[nt_ids], :40] = scaled_li[nt_ids].astype(np.float16)
    tbl_t1_g = np.zeros((tblt_rows, TPAD), dtype=np.float16)
    tbl_t1_g[tix[t_ids], :40] = scaled_li[t_ids].astype(np.float16)
    tbl_t2_g = np.zeros((tblt_rows, TPAD), dtype=np.float16)
    tbl_t2_g[tix[t_ids], :40] = scaled_hd[t_ids].astype(np.float16)

    # ---- per-core MLP / combine inputs ----
    inv = np.full(nt_pad, -1, dtype=np.int64)
    inv[pid[nt_ids]] = nt_ids
    nt_map = inv.reshape(NCORES, s_pad)

    def stripe_rows(ids, srl, spad_, nstripes=NCORES):
        m = np.full((nstripes, spad_), -1, dtype=np.int64)
        for i in range(nstripes):
            lo = i * srl
            hi = min(len(ids), (i + 1) * srl)
            if hi > lo:
                m[i, : hi - lo] = ids[lo:hi]
        return m

    t_map = stripe_rows(t_ids, st_real, st_pad)

    def take(arr, idmap, fill=0.0):
        out = np.full((idmap.shape[0], idmap.shape[1]) + arr.shape[1:], fill,
                      dtype=arr.dtype)
        valid = idmap >= 0
        out[valid] = arr[idmap[valid]]
        return out

    xnt_g = np.ascontiguousarray(
        take(x, nt_map).astype(np.float16).transpose(0, 2, 1)
    )
    xt_g = np.ascontiguousarray(take(x, t_map).astype(np.float16).transpose(0, 2, 1))
    al_nt_g = take(alpha, nt_map).astype(np.float32)
    al_t_g = take(alpha, t_map).astype(np.float32)
    dis_nt_g = take(dis[:, None], nt_map).astype(np.float32)
    dsq_nt_g = take((dis * dis)[:, None], nt_map).astype(np.float32)
    hard_t_g = take(hard, t_map).astype(np.float32)

    w1t_g = fc1_w.T.astype(np.float16).copy()  # [512, 256]
    b1_g = fc1_b.reshape(256, 1).astype(np.float32)
    w2t_g = fc2_w.T.astype(np.float16).copy()  # [256, 40]
    b2b_g = np.tile(fc2_b.reshape(1, 40), (P, 1)).astype(np.float32)

    nc = _build_program(pm, pc, s_pad, st_pad, tn, tt, nt_pad, tblt_rows)

    if os.environ.get("KERNEL_BUILD_ONLY", "0") == "1":
        e = BuildOnly()
        e.nc = nc
        raise e

    in_maps = []
    for i in range(NCORES):
        in_maps.append(
            dict(
                tbl_init=tbl_init_g,
                tbl_t1=tbl_t1_g,
                tbl_t2=tbl_t2_g,
                idx_nt=pm.wrapped_idx(i),
                idx_t=pc.wrapped_idx(i),
                xnt=xnt_g[i],
                xt=xt_g[i],
                w1t=w1t_g,
                b1=b1_g,
                w2t=w2t_g,
                b2b=b2b_g,
                alpha_nt=al_nt_g[i],
                alpha_t=al_t_g[i],
                dis_nt=dis_nt_g[i],
                dissq_nt=dsq_nt_g[i],
                hard_t=hard_t_g[i],
            )
        )

    if os.environ.get("KERNEL_SIM", "0") == "1":
        from concourse import bass_interp

        sim = bass_interp.MultiCoreSim(nc, NCORES)
        for i in range(NCORES):
            for k, v in in_maps[i].items():
                sim.cores[i].tensor(k)[:] = v
        sim.simulate()
        results = [
            {k: np.array(sim.cores[i].mem_tensor(k)) for k in ("out_nt", "out_t")}
            for i in range(NCORES)
        ]
        res = None
    else:
        res = run_bass_kernel_spmd(
            nc, in_maps, core_ids=list(range(NCORES)),
            trace=bool(int(os.environ.get("KERNEL_TRACE", "0"))),
        )
        results = res.results
    kernel.last_results = res
    kernel.last_nc = nc
    kernel.last_in_maps = in_maps

    out = np.zeros((n, 40), dtype=np.float32)
    for i in range(NCORES):
        om = results[i]["out_nt"]
        ot = results[i]["out_t"]
        v = nt_map[i] >= 0
        out[nt_map[i][v]] = om[v]
        v = t_map[i] >= 0
        out[t_map[i][v]] = ot[v]
    return out


# revision 22
# speedup vs baseline: 2.6186x; 1.0025x over previous
"""CPFStudent (GNN label propagation + MLP mix) on 8 TRN2 NeuronCores.

Strategy v2 (column-gather + vector-engine segment reduce):
  - Reference: 10 PLP steps of plp <- where(mask, hard, A_hat @ plp), with
    A_hat = D^-1/2 (A+I) D^-1/2 built from out-degrees of edge_index[0];
    final logits = sigmoid(alpha)*plp + (1-sigmoid(alpha))*relu(x@W1^T+b1)@W2^T+b2.
  - Only non-train (NT) rows of plp evolve.  State kept as table = dis * plp
    (dis = deg^-1/2), fp16, 256B-strided rows in HBM:
        plp_new[d] = dis[d] * ( sum_{e: src NT} table[src] + c[d] )
    c from T sources is constant (c1 for step 1 from label_init, c2 after).
  - NT nodes are sorted by in-degree and dealt into 128-node tiles; tiles are
    assigned to cores round-robin so per-core load balances and tiles are
    degree-homogeneous.
  - Per dst tile, in-edges are laid out COLUMN-major: gather writes message
    j of dst d to [partition d, column j]; short dsts padded with a known
    zero table row.  Segment sum = ONE strided tensor_reduce per tile on the
    vector engine (no selector matmuls, no S blobs) — this kernel is
    instruction-dispatch-bound on real HW, so instruction count is king.
  - Sources split in 2 chunks so gather indices fit int16.
  - Per-iteration halo exchange: AllGather of compact fp16 rows, then a
    strided DMA expands them into the 256B-strided table.
"""

import math
import os
import sys

import numpy as np

sys.path.insert(0, "/opt/trn_rl_repo")

import concourse.bass as bass  # noqa: E402
import concourse.mybir as mybir  # noqa: E402
import concourse.tile as tile  # noqa: E402
from concourse import bacc  # noqa: E402
from concourse.bass_utils import run_bass_kernel_spmd  # noqa: E402

P = 128
NCORES = 8
TPAD = 128  # fp16 elements per table row (256B, dma_gather elem granularity)
MAX_CALL = int(os.environ.get("KERNEL_MAX_CALL", "1024"))
SCRATCH = int(os.environ.get("KERNEL_SCRATCH", "16384"))
NQUEUES = int(os.environ.get("KERNEL_QUEUES", "1"))

F16 = mybir.dt.float16
F32 = mybir.dt.float32
I16 = mybir.dt.int16


def _ceil(a, b):
    return -(-a // b)


class BuildOnly(Exception):
    pass


class ColPlan:
    """Column-major per-dst edge layout for one gather+reduce pass.

    src_row: global row index into the pass's source table
    dst_pid: padded NT id of the destination
    Layout (identical across cores): tile-major; per tile, chunk-0 columns
    then chunk-1 columns; within a (tile, chunk) block column j of dst slot p
    sits at flat position block_off + j*128 + p.  Unused slots hold the
    chunk's known-zero row.
    """

    def __init__(self, src_row, dst_pid, chunk, n_chunks, zero_rel, s_pad, tn):
        self.chunk = chunk
        self.n_chunks = n_chunks
        self.tn = tn

        core = dst_pid // s_pad
        dloc = dst_pid % s_pad
        tl = dloc // P
        slot = dloc % P
        ch = src_row // chunk
        rel = src_row - ch * chunk

        key = ((core * tn + tl) * n_chunks + ch) * P + slot
        counts = np.bincount(
            key, minlength=NCORES * tn * n_chunks * P
        ).reshape(NCORES, tn, n_chunks, P)
        caps = counts.max(axis=(0, 3))  # [tn, n_chunks]
        self.caps = caps
        self.cols_per_tile = caps.sum(axis=1)  # [tn]

        blk = caps * P
        flat_off = np.zeros((tn, n_chunks), dtype=np.int64)
        off = 0
        for t in range(tn):
            for c in range(n_chunks):
                flat_off[t, c] = off
                off += int(blk[t, c])
        self.total_idx = off
        # idx-stream offset of each tile's first block (+ total sentinel)
        self.tile_off = np.concatenate([flat_off[:, 0], [off]])

        idx = np.empty((NCORES, off), dtype=np.int16)
        for t in range(tn):
            for c in range(n_chunks):
                idx[:, flat_off[t, c] : flat_off[t, c] + blk[t, c]] = zero_rel[c]

        # rank of each edge within its (core, tile, chunk, slot) bucket
        order = np.argsort(key, kind="stable")
        key_o = key[order]
        uniq, inv, cnt = np.unique(key_o, return_inverse=True, return_counts=True)
        starts = np.concatenate([[0], np.cumsum(cnt)])[:-1]
        rank_o = np.arange(len(key_o)) - starts[inv]
        rank = np.empty_like(rank_o)
        rank[order] = rank_o

        pos = flat_off[tl, ch] + rank * P + slot
        idx[core, pos] = rel.astype(np.int16)
        self.idx16 = idx

        # calls: per (tile, chunk) block, subcalls of <= MAX_CALL idxs
        self.calls = {t: [] for t in range(tn)}  # (chunk, idx_off, n, col0)
        self.n_calls = 0
        for t in range(tn):
            col0 = 0
            for c in range(n_chunks):
                n = int(blk[t, c])
                base = int(flat_off[t, c])
                p0 = 0
                while p0 < n:
                    k = min(MAX_CALL, n - p0)
                    self.calls[t].append((c, base + p0, k, col0 + p0 // P))
                    self.n_calls += 1
                    p0 += k
                col0 += int(caps[t, c])

    def wrapped_idx(self, core):
        """[128, total_idx//16] int16, wrapped-16 and replicated to 8 groups."""
        v = self.idx16[core].reshape(-1, 16).T  # [16, total/16]
        return np.tile(v, (8, 1)).copy()


def _build_program(pm, pc, s_pad, st_pad, tn, tt, tbl_rows, tblt_rows):
    nt_pad = NCORES * s_pad
    nc = bacc.Bacc(
        None,
        target_bir_lowering=False,
        num_devices=NCORES,
        dynamic_dma_scratch_size=SCRATCH,
        num_swdge_queues=NQUEUES,
    )

    def param(name, shape, dt, out=False):
        return nc.declare_dram_parameter(name, list(shape), dt, isOutput=out)

    tbl_init = param("tbl_init", (tbl_rows, TPAD), F16)
    tbl_t1 = param("tbl_t1", (tblt_rows, TPAD), F16)
    tbl_t2 = param("tbl_t2", (tblt_rows, TPAD), F16)
    idx_nt = param("idx_nt", (P, pm.total_idx // 16), I16)
    idx_t = param("idx_t", (P, pc.total_idx // 16), I16)
    xnt = param("xnt", (512, s_pad), F16)  # pre-transposed on host
    xt = param("xt", (512, st_pad), F16)
    w1t = param("w1t", (512, 256), F16)
    b1 = param("b1", (256, 1), F32)
    w2t = param("w2t", (256, 40), F16)
    b2b = param("b2b", (P, 40), F32)
    alpha_nt = param("alpha_nt", (s_pad, 1), F32)
    alpha_t = param("alpha_t", (st_pad, 1), F32)
    dis_nt = param("dis_nt", (s_pad, 1), F32)
    dissq_nt = param("dissq_nt", (s_pad, 1), F32)
    hard_t = param("hard_t", (st_pad, 40), F32)
    out_nt = param("out_nt", (s_pad, 40), F32, out=True)
    out_t = param("out_t", (st_pad, 40), F32, out=True)

    table = nc.dram_tensor("table", [tbl_rows, TPAD], F16)
    # ping-pong the collective in/out buffers: a lagging peer may still be
    # pulling iteration k's data after our collective instruction completed,
    # so iteration k+1 must not overwrite the same buffers
    cown = [nc.dram_tensor(f"cown{i}", [s_pad, 40], F16) for i in range(2)]
    callg = [
        nc.dram_tensor(f"callg{i}", [nt_pad, 40], F16, addr_space="Shared")
        for i in range(2)
    ]

    RG = [list(range(NCORES))]
    CH_M = pm.chunk
    CH_T = pc.chunk

    with tile.TileContext(nc) as tc:
        with (
            tc.tile_pool(name="persist", bufs=1) as pp,
            tc.tile_pool(name="work", bufs=4) as wp,
            tc.tile_pool(name="bigtmp", bufs=1) as bp,
            tc.tile_pool(name="gpool", bufs=4) as gp,
            tc.tile_pool(name="mpsum", bufs=2, space="PSUM") as mp,
            tc.tile_pool(name="mpsum2", bufs=4, space="PSUM") as mp2,
        ):
            # one-time init: fills pad columns so later strided updates leave
            # only finite data for gathers
            nc.sync.dma_start(out=table[:, :], in_=tbl_init[:, :])

            # ---- persistent SBUF ----
            idxm_sb = pp.tile([P, pm.total_idx // 16], I16, tag="idxm")
            nc.sync.dma_start(out=idxm_sb[:], in_=idx_nt[:, :])

            ft_nt = pp.tile([P, tn, 40], F32, tag="ftnt")
            ft_t = pp.tile([P, tt, 40], F32, tag="ftt")
            c1 = pp.tile([P, tn, 40], F32, tag="c1")
            c2 = pp.tile([P, tn, 40], F32, tag="c2")
            sums = pp.tile([P, tn, 40], F32, tag="sums")
            compact = pp.tile([P, tn, 40], F16, tag="compact")
            # pad tiles are skipped by every pass but their rows flow to the
            # table via cown/callg each iteration — they must stay zero (they
            # are the gather-padding zero rows), and the batched evac reads
            # whole buffers, so zero everything once
            nc.vector.memset(compact[:], 0.0)
            nc.vector.memset(sums[:], 0.0)
            nc.vector.memset(c1[:], 0.0)
            nc.vector.memset(c2[:], 0.0)

            def bc40(ap):
                """[P, n] -> [P, n, 40] stride-0 broadcast view."""
                return bass.AP(ap.tensor, ap.offset, [*ap.ap, [0, 40]])

            w1_sb = pp.tile([P, 4, 256], F16, tag="w1")
            nc.sync.dma_start(
                out=w1_sb[:], in_=w1t.ap().rearrange("(k p) h -> p k h", p=P)
            )
            w2_sb = pp.tile([P, 2, 40], F16, tag="w2")
            nc.sync.dma_start(
                out=w2_sb[:], in_=w2t.ap().rearrange("(h p) c -> p h c", p=P)
            )
            b1_sb = pp.tile([P, 2], F32, tag="b1")
            nc.sync.dma_start(
                out=b1_sb[:], in_=b1.ap().rearrange("(h p) o -> p (h o)", p=P)
            )
            b2_sb = pp.tile([P, 40], F32, tag="b2")
            nc.sync.dma_start(out=b2_sb[:], in_=b2b[:, :])

            def cols_load(prm, n_tiles, tag):
                t_ = pp.tile([P, n_tiles], F32, tag=tag)
                nc.sync.dma_start(
                    out=t_[:], in_=prm.ap().rearrange("(t p) o -> p (t o)", p=P)
                )
                return t_

            disn_sb = cols_load(dis_nt, tn, "disn")
            dsqn_sb = cols_load(dissq_nt, tn, "dsqn")
            aln_sb = cols_load(alpha_nt, tn, "aln")
            alt_sb = cols_load(alpha_t, tt, "alt")

            # sigmoid(alpha); a*dis; 1-a
            sign_sb = pp.tile([P, tn], F32, tag="sign")
            nc.scalar.activation(
                sign_sb[:], aln_sb[:], mybir.ActivationFunctionType.Sigmoid
            )
            sigt_sb = pp.tile([P, tt], F32, tag="sigt")
            nc.scalar.activation(
                sigt_sb[:], alt_sb[:], mybir.ActivationFunctionType.Sigmoid
            )
            disa_sb = pp.tile([P, tn], F32, tag="disa")
            nc.vector.tensor_tensor(
                out=disa_sb[:], in0=sign_sb[:], in1=disn_sb[:],
                op=mybir.AluOpType.mult,
            )
            oman_sb = pp.tile([P, tn], F32, tag="oman")
            nc.vector.tensor_scalar(
                out=oman_sb[:], in0=sign_sb[:], scalar1=-1.0, scalar2=1.0,
                op0=mybir.AluOpType.mult, op1=mybir.AluOpType.add,
            )
            omat_sb = pp.tile([P, tt], F32, tag="omat")
            nc.vector.tensor_scalar(
                out=omat_sb[:], in0=sigt_sb[:], scalar1=-1.0, scalar2=1.0,
                op0=mybir.AluOpType.mult, op1=mybir.AluOpType.add,
            )

            # ---- MLP (FT branch), 4 node-tiles per matmul group ----
            def mlp(xsrc, n_tiles, ft_dst):
                g0 = 0
                while g0 < n_tiles:
                    gw = min(4, n_tiles - g0)
                    W = gw * P
                    xT4s = []
                    for k in range(4):
                        xT4 = wp.tile([P, W], F16, tag="xT4")
                        nc.sync.dma_start(
                            out=xT4[:],
                            in_=xsrc[k * P : (k + 1) * P, g0 * P : g0 * P + W],
                        )
                        xT4s.append(xT4)
                    ps2s = [
                        mp2.tile([P, 40], F32, tag="ps2", name=f"ps2_{j}")
                        for j in range(gw)
                    ]
                    for h in range(2):
                        ps1 = mp.tile([P, W], F32, tag="ps1")
                        for k in range(4):
                            nc.tensor.matmul(
                                ps1[:],
                                lhsT=w1_sb[:, k, h * P : (h + 1) * P],
                                rhs=xT4s[k][:],
                                start=(k == 0),
                                stop=(k == 3),
                            )
                        hT = wp.tile([P, W], F16, tag="hT4")
                        nc.scalar.activation(
                            hT[:], ps1[:], mybir.ActivationFunctionType.Relu,
                            bias=b1_sb[:, h : h + 1],
                        )
                        for j in range(gw):
                            nc.tensor.matmul(
                                ps2s[j][:],
                                lhsT=hT[:, j * P : (j + 1) * P],
                                rhs=w2_sb[:, h, :],
                                start=(h == 0), stop=(h == 1),
                            )
                    for j in range(gw):
                        nc.vector.tensor_tensor(
                            out=ft_dst[:, g0 + j, :], in0=ps2s[j][:],
                            in1=b2_sb[:], op=mybir.AluOpType.add,
                        )
                    g0 += gw

            mlp(xnt, tn, ft_nt)
            mlp(xt, tt, ft_t)

            # ---- generic column-gather + reduce pass ----
            _regs = {}

            def num_reg(v):
                if v not in _regs:
                    _regs[v] = nc.gpsimd.to_reg(v)
                return _regs[v]

            def col_pass(plan, tsrc, idx_sb, ch_rows, dst,
                         t_lo=0, t_hi=None, idx_base=0):
                """Gathers per dst tile + one strided reduce into dst[:, t, :].

                t_lo/t_hi restrict to a tile range; idx_base is the stream
                offset of idx_sb[0] within the plan's flat idx stream."""
                qi = 0
                if t_hi is None:
                    t_hi = plan.tn
                for t in range(t_lo, t_hi):
                    cols = int(plan.cols_per_tile[t])
                    if cols == 0:
                        continue
                    gb = gp.tile([P, cols, TPAD], F16, tag="gb")
                    for c, off, n_, col0 in plan.calls[t]:
                        off -= idx_base
                        nc.gpsimd.dma_gather(
                            out_ap=gb[:, col0 : col0 + n_ // P, :],
                            in_ap=tsrc[c * ch_rows : (c + 1) * ch_rows, :],
                            idxs_ap=idx_sb[:, off // 16 : (off + n_) // 16],
                            num_idxs=n_,
                            num_idxs_reg=num_reg(n_),
                            elem_size=TPAD,
                            queue_num=qi % NQUEUES,
                        )
                        qi += 1
                    nc.vector.tensor_reduce(
                        out=dst[:, t, :],
                        in_=gb[:, :, 0:40].rearrange("p n e -> p e n"),
                        axis=mybir.AxisListType.X,
                        op=mybir.AluOpType.add,
                    )

            # ---- c1 / c2 passes (T sources) ----
            # split tiles at ~half the idx stream so the buffer stays small
            t_mid = pc.tn // 2
            while pc.tile_off[t_mid] % 16 != 0:
                t_mid += 1
            splits = [(0, t_mid), (t_mid, pc.tn)]
            buf_cols = max(
                (pc.tile_off[hi] - pc.tile_off[lo]) // 16 for lo, hi in splits
            )
            with tc.tile_pool(name="cidx", bufs=1) as cp:
                for lo, hi in splits:
                    base = int(pc.tile_off[lo])
                    ncols = (int(pc.tile_off[hi]) - base) // 16
                    if ncols == 0:
                        continue
                    idxc_sb = cp.tile([P, buf_cols], I16, tag="idxc")
                    nc.sync.dma_start(
                        out=idxc_sb[:, 0:ncols],
                        in_=idx_t[:, base // 16 : base // 16 + ncols],
                    )
                    col_pass(pc, tbl_t1, idxc_sb, CH_T, c1, lo, hi, base)
                    col_pass(pc, tbl_t2, idxc_sb, CH_T, c2, lo, hi, base)

            # ---- 10 PLP iterations ----
            for it in range(10):
                tsrc = tbl_init if it == 0 else table
                cbuf = c1 if it == 0 else c2

                col_pass(pm, tsrc, idxm_sb, CH_M, sums)

                if it < 9:
                    # compact = dsq * (sums + c); batched over all tiles
                    tmp = bp.tile([P, tn, 40], F32, tag="ev")
                    nc.vector.tensor_tensor(
                        out=tmp[:], in0=sums[:], in1=cbuf[:],
                        op=mybir.AluOpType.add,
                    )
                    nc.vector.tensor_tensor(
                        out=compact[:], in0=tmp[:], in1=bc40(dsqn_sb[:]),
                        op=mybir.AluOpType.mult,
                    )
                else:
                    # out = a*dis*(sums + c) + (1-a)*ft; batched, in place
                    tmp = bp.tile([P, tn, 40], F32, tag="ev")
                    nc.vector.tensor_tensor(
                        out=tmp[:], in0=sums[:], in1=cbuf[:],
                        op=mybir.AluOpType.add,
                    )
                    nc.vector.tensor_tensor(
                        out=tmp[:], in0=tmp[:], in1=bc40(disa_sb[:]),
                        op=mybir.AluOpType.mult,
                    )
                    t3 = bp.tile([P, tn, 40], F32, tag="ev3")
                    nc.vector.tensor_tensor(
                        out=t3[:], in0=ft_nt[:], in1=bc40(oman_sb[:]),
                        op=mybir.AluOpType.mult,
                    )
                    nc.vector.tensor_tensor(
                        out=tmp[:], in0=tmp[:], in1=t3[:],
                        op=mybir.AluOpType.add,
                    )
                    nc.sync.dma_start(
                        out=out_nt.ap().rearrange("(t p) c -> p t c", p=P),
                        in_=tmp[:],
                    )

                if it < 9:
                    cw, cg = cown[it % 2], callg[it % 2]
                    nc.sync.dma_start(
                        out=cw.ap().rearrange("(t p) c -> p t c", p=P),
                        in_=compact[:],
                    )
                    if os.environ.get("KERNEL_NO_CC", "0") == "1":
                        # debug mode: skip the collective (wrong cross-core data)
                        nc.sync.dma_start(
                            out=cg[0 : s_pad, :], in_=cw[:, :]
                        )
                    else:
                        nc.gpsimd.collective_compute(
                            "AllGather",
                            mybir.AluOpType.bypass,
                            replica_groups=RG,
                            ins=[cw.ap().opt()],
                            outs=[cg.ap().opt()],
                        )
                    nc.sync.dma_start(out=table[0:nt_pad, 0:40], in_=cg[:, :])

            # ---- T-side final combine (batched) ----
            hsb = bp.tile([P, tt, 40], F32, tag="ev")
            nc.sync.dma_start(
                out=hsb[:], in_=hard_t.ap().rearrange("(t p) c -> p t c", p=P)
            )
            nc.vector.tensor_tensor(
                out=hsb[:], in0=hsb[:], in1=bc40(sigt_sb[:]),
                op=mybir.AluOpType.mult,
            )
            t2_ = bp.tile([P, tt, 40], F32, tag="ev3")
            nc.vector.tensor_tensor(
                out=t2_[:], in0=ft_t[:], in1=bc40(omat_sb[:]),
                op=mybir.AluOpType.mult,
            )
            nc.vector.tensor_tensor(
                out=hsb[:], in0=hsb[:], in1=t2_[:],
                op=mybir.AluOpType.add,
            )
            nc.sync.dma_start(
                out=out_t.ap().rearrange("(t p) c -> p t c", p=P), in_=hsb[:]
            )

    nc.compile()
    return nc


def kernel(**inputs):
    x = np.asarray(inputs["x"], dtype=np.float32)
    edge_index = np.asarray(inputs["edge_index"])
    label_init = np.asarray(inputs["label_init"], dtype=np.float32)
    train_mask = np.asarray(inputs["train_mask"]).astype(bool)
    hard = np.asarray(inputs["hard_one_hot"], dtype=np.float32)
    fc1_w = np.asarray(inputs["fc1_w"], dtype=np.float32)
    fc1_b = np.asarray(inputs["fc1_b"], dtype=np.float32)
    fc2_w = np.asarray(inputs["fc2_w"], dtype=np.float32)
    fc2_b = np.asarray(inputs["fc2_b"], dtype=np.float32)
    alpha = np.asarray(inputs["alpha"], dtype=np.float32)

    n = x.shape[0]
    row = edge_index[0].astype(np.int64)
    col = edge_index[1].astype(np.int64)

    deg = np.bincount(row, minlength=n).astype(np.float64) + 1.0
    dis = (1.0 / np.sqrt(deg)).astype(np.float32)

    nt_ids = np.nonzero(~train_mask)[0]
    t_ids = np.nonzero(train_mask)[0]
    n_nt, n_t = len(nt_ids), len(t_ids)

    # tiles of 128 NT nodes, degree-sorted, dealt round-robin to cores.
    # +1 tile rank per core so every core ends with an all-pad (zero) tile,
    # giving each chunk a known zero row for gather padding.
    n_tiles_real = _ceil(n_nt, P)
    tn = _ceil(n_tiles_real, NCORES) + 1
    s_pad = tn * P
    nt_pad = NCORES * s_pad
    assert nt_pad % 2 == 0
    CH_M = nt_pad // 2  # main-table chunk rows

    st_real = _ceil(n_t, NCORES)
    tt = _ceil(st_real, P)
    st_pad = tt * P

    # in-degree of each node counting NT sources + self loop (what the main
    # pass pads against)
    sel_nt_src = ~train_mask[row]
    deg_in_nt = np.bincount(col[sel_nt_src], minlength=n) + 1

    order = np.argsort(-deg_in_nt[nt_ids], kind="stable")  # NT rank by degree
    ranked = nt_ids[order]  # i-th highest-degree NT node
    i_arr = np.arange(n_nt)
    k_tile = i_arr // P
    core_of = k_tile % NCORES
    tin_of = k_tile // NCORES
    pid = np.full(n, -1, dtype=np.int64)
    pid[ranked] = core_of * s_pad + tin_of * P + (i_arr % P)

    # main zero row: every core's last tile is all-pad; cores 0 and NCORES//2
    # start chunks 0/1, so relative offset (tn-1)*P is zero in both chunks
    zero_m = (tn - 1) * P
    assert zero_m < CH_M

    # T table: row 0 is a zero row, rows 1..n_t real; 2 chunks; chunk 1 must
    # end with at least one zero (pad) row
    tblt_rows_min = n_t + 1
    CH_T = _ceil(tblt_rows_min, 2)
    if 2 * CH_T == tblt_rows_min:
        CH_T += 64
    tblt_rows = 2 * CH_T
    assert tblt_rows > tblt_rows_min
    zero_t = (0, max(tblt_rows_min - CH_T, 0))  # per-chunk zero row (relative)
    assert 0 <= zero_t[1] < CH_T
    tix = np.full(n, -1, dtype=np.int64)
    tix[t_ids] = 1 + np.arange(n_t)

    # edges into NT dsts
    sel = ~train_mask[col]
    es, ed = row[sel], col[sel]
    src_nt = ~train_mask[es]
    # main: NT->NT plus self-loops on NT
    m_src = np.concatenate([pid[es[src_nt]], pid[nt_ids]])
    m_dst = np.concatenate([pid[ed[src_nt]], pid[nt_ids]])
    pm = ColPlan(m_src, m_dst, CH_M, 2, (zero_m, zero_m), s_pad, tn)
    # cpass: T->NT
    c_src = tix[es[~src_nt]]
    c_dst = pid[ed[~src_nt]]
    pc = ColPlan(c_src, c_dst, CH_T, 2, zero_t, s_pad, tn)

    # ---- tables ----
    scaled_li = dis[:, None] * label_init  # [n, 40]
    scaled_hd = dis[:, None] * hard

    tbl_init_g = np.zeros((nt_pad, TPAD), dtype=np.float16)
    tbl_init_g[pid[nt_ids], :40] = scaled_li[nt_ids].astype(np.float16)
    tbl_t1_g = np.zeros((tblt_rows, TPAD), dtype=np.float16)
    tbl_t1_g[tix[t_ids], :40] = scaled_li[t_ids].astype(np.float16)
    tbl_t2_g = np.zeros((tblt_rows, TPAD), dtype=np.float16)
    tbl_t2_g[tix[t_ids], :40] = scaled_hd[t_ids].astype(np.float16)

    # ---- per-core MLP / combine inputs ----
    inv = np.full(nt_pad, -1, dtype=np.int64)
    inv[pid[nt_ids]] = nt_ids
    nt_map = inv.reshape(NCORES, s_pad)

    def stripe_rows(ids, srl, spad_, nstripes=NCORES):
        m = np.full((nstripes, spad_), -1, dtype=np.int64)
        for i in range(nstripes):
            lo = i * srl
            hi = min(len(ids), (i + 1) * srl)
            if hi > lo:
                m[i, : hi - lo] = ids[lo:hi]
        return m

    t_map = stripe_rows(t_ids, st_real, st_pad)

    def take(arr, idmap, fill=0.0):
        out = np.full((idmap.shape[0], idmap.shape[1]) + arr.shape[1:], fill,
                      dtype=arr.dtype)
        valid = idmap >= 0
        out[valid] = arr[idmap[valid]]
        return out

    xnt_g = np.ascontiguousarray(
        take(x, nt_map).astype(np.float16).transpose(0, 2, 1)
    )
    xt_g = np.ascontiguousarray(take(x, t_map).astype(np.float16).transpose(0, 2, 1))
    al_nt_g = take(alpha, nt_map).astype(np.float32)
    al_t_g = take(alpha, t_map).astype(np.float32)
    dis_nt_g = take(dis[:, None], nt_map).astype(np.float32)
    dsq_nt_g = take((dis * dis)[:, None], nt_map).astype(np.float32)
    hard_t_g = take(hard, t_map).astype(np.float32)

    w1t_g = fc1_w.T.astype(np.float16).copy()  # [512, 256]
    b1_g = fc1_b.reshape(256, 1).astype(np.float32)
    w2t_g = fc2_w.T.astype(np.float16).copy()  # [256, 40]
    b2b_g = np.tile(fc2_b.reshape(1, 40), (P, 1)).astype(np.float32)

    nc = _build_program(pm, pc, s_pad, st_pad, tn, tt, nt_pad, tblt_rows)

    if os.environ.get("KERNEL_BUILD_ONLY", "0") == "1":
        e = BuildOnly()
        e.nc = nc
        raise e

    in_maps = []
    for i in range(NCORES):
        in_maps.append(
            dict(
                tbl_init=tbl_init_g,
                tbl_t1=tbl_t1_g,
                tbl_t2=tbl_t2_g,
                idx_nt=pm.wrapped_idx(i),
                idx_t=pc.wrapped_idx(i),
                xnt=xnt_g[i],
                xt=xt_g[i],
                w1t=w1t_g,
                b1=b1_g,
                w2t=w2t_g,
                b2b=b2b_g,
                alpha_nt=al_nt_g[i],
                alpha_t=al_t_g[i],
                dis_nt=dis_nt_g[i],
                dissq_nt=dsq_nt_g[i],
                hard_t=hard_t_g[i],
            )
        )

    if os.environ.get("KERNEL_SIM", "0") == "1":
        from concourse import bass_interp

        sim = bass_interp.MultiCoreSim(nc, NCORES)
        for i in range(NCORES):
            for k, v in in_maps[i].items():
                sim.cores[i].tensor(k)[:] = v
        sim.simulate()
        results = [
            {k: np.array(sim.cores[i].mem_tensor(k)) for k in ("out_nt", "out_t")}
            for i in range(NCORES)
        ]
        res = None
    else:
        res = run_bass_kernel_spmd(
            nc, in_maps, core_ids=list(range(NCORES)),
            trace=bool(int(os.environ.get("KERNEL_TRACE", "0"))),
        )
        results = res.results
    kernel.last_results = res
    kernel.last_nc = nc
    kernel.last_in_maps = in_maps

    out = np.zeros((n, 40), dtype=np.float32)
    for i in range(NCORES):
        om = results[i]["out_nt"]
        ot = results[i]["out_t"]
        v = nt_map[i] >= 0
        out[nt_map[i][v]] = om[v]
        v = t_map[i] >= 0
        out[t_map[i][v]] = ot[v]
    return out


# revision 24
# speedup vs baseline: 3.6964x; 1.4116x over previous
"""CPFStudent (GNN label propagation + MLP mix) on 8 TRN2 NeuronCores.

Strategy (dst-sharded SpMM with selector matmuls):
  - Reference: 10 PLP steps of plp <- where(mask, hard, A_hat @ plp), with
    A_hat = D^-1/2 (A+I) D^-1/2 built from out-degrees of edge_index[0];
    final logits = sigmoid(alpha)*plp + (1-sigmoid(alpha))*relu(x@W1^T+b1)@W2^T+b2.
  - Only non-train (NT) rows of plp evolve; train (T) rows are constant after
    step 1.  We keep the state as table = dis * plp (dis = deg^-1/2), fp16,
    so per-edge messages need no norm multiply:
        plp_new[d] = dis[d] * ( sum_{e: src NT} table[src] + c )
    where c is a constant per dst: c1 (from dis*label_init over T srcs, used in
    step 1) or c2 (from dis*hard over T srcs, steps 2..10).
  - Nodes are permuted host-side: NT nodes first, padded per-core stripes.
    Each core owns a contiguous stripe of NT dst rows; edges are bucketed by
    (dst_tile of 128, src chunk of <=32768 rows) host-side, padded to uniform
    capacities across cores (SPMD), and gathered per iteration with
    gpsimd.dma_gather (256B elements) from an HBM fp16 table.
  - Scatter/segment-sum is done on the TensorEngine: per 128-edge slot a
    host-precomputed fp8 selector S (S[e,d]=1 iff dst_local(e)==d) multiplies
    the gathered messages, accumulating in PSUM per dst tile.
  - Per-iteration halo exchange: AllGather of each core's new compact fp16
    rows, then a strided DMA expands them into the 256B-strided table.
"""

import math
import os
import sys

import numpy as np

sys.path.insert(0, "/opt/trn_rl_repo")

import ml_dtypes  # noqa: E402

import concourse.bass as bass  # noqa: E402
import concourse.mybir as mybir  # noqa: E402
import concourse.tile as tile  # noqa: E402
from concourse import bacc  # noqa: E402
from concourse.bass_utils import run_bass_kernel_spmd  # noqa: E402

P = 128
NCORES = 8
TPAD = 128  # fp16 elements per table row (256B, dma_gather elem granularity)
GROUP = 7  # dst tiles per dma_gather call group
MAX_CALL = int(os.environ.get("KERNEL_MAX_CALL", "1024"))

F16 = mybir.dt.float16
F32 = mybir.dt.float32
F8 = mybir.dt.float8e4
I16 = mybir.dt.int16
NP_F8 = ml_dtypes.float8_e4m3


def _ceil(a, b):
    return -(-a // b)


class BuildOnly(Exception):
    pass


class EdgePlan:
    """Host-side bucketed edge plan for one SpMM pass, uniform across cores.

    src_row: int array, row index into the pass's gather table
    dst_pid: int array, padded NT id of the destination
    """

    def __init__(self, src_row, dst_pid, n_rows, s_pad, n_tiles):
        self.n_chunks = max(1, _ceil(n_rows, 32768))
        self.chunk = _ceil(n_rows, self.n_chunks)
        self.n_tiles = n_tiles
        nch = self.n_chunks

        core = dst_pid // s_pad
        dloc = dst_pid - core * s_pad
        tl = dloc // P
        dstloc = dloc % P
        ch = src_row // self.chunk

        key = (core * n_tiles + tl) * nch + ch
        counts = np.bincount(key, minlength=NCORES * n_tiles * nch).reshape(
            NCORES, n_tiles, nch
        )
        caps = counts.max(axis=0)  # [n_tiles, nch]
        caps = ((caps + P - 1) // P) * P
        self.caps = caps
        self.slots_per_tile = caps.sum(axis=1) // P  # [n_tiles]
        self.s_off = np.concatenate([[0], np.cumsum(self.slots_per_tile)])
        self.total_slots = int(self.s_off[-1])

        # per (chunk, group) call: num idxs and per-tile column offsets
        self.n_groups = _ceil(n_tiles, GROUP)
        self.call_num = np.zeros((nch, self.n_groups), dtype=np.int64)
        self.buck_col = np.zeros((nch, n_tiles), dtype=np.int64)  # col in its call buf
        for c in range(nch):
            for g in range(self.n_groups):
                off = 0
                for t in range(g * GROUP, min((g + 1) * GROUP, n_tiles)):
                    self.buck_col[c, t] = off
                    off += caps[t, c] // P
                self.call_num[c, g] = off * P
        # col offset of each call inside the flat idx stream (per chunk then group)
        self.call_off = np.zeros((nch, self.n_groups), dtype=np.int64)
        off = 0
        for c in range(nch):
            for g in range(self.n_groups):
                self.call_off[c, g] = off
                off += self.call_num[c, g]
        self.total_idx = off

        # sub-calls of <= MAX_CALL idxs: per (c, g) a list of (idx_off, num, col0)
        self.subcalls = {}
        for c in range(nch):
            for g in range(self.n_groups):
                num = int(self.call_num[c, g])
                base = int(self.call_off[c, g])
                subs = []
                p0 = 0
                while p0 < num:
                    n_ = min(MAX_CALL, num - p0)
                    subs.append((base + p0, n_, p0 // P))
                    p0 += n_
                self.subcalls[(c, g)] = subs

        # order edges by (core, chunk, tile); build padded per-core streams
        order = np.argsort((core * nch + ch) * n_tiles + tl, kind="stable")
        src_o = src_row[order]
        core_o = core[order]
        ch_o = ch[order]
        tl_o = tl[order]
        dst_o = dstloc[order]

        # destination position of each edge in the padded stream
        # padded stream order: for chunk c, group g, tile t in g: cap[t,c] entries
        base_tc = np.zeros((nch, n_tiles), dtype=np.int64)
        for c in range(nch):
            for g in range(self.n_groups):
                for t in range(g * GROUP, min((g + 1) * GROUP, n_tiles)):
                    base_tc[c, t] = self.call_off[c, g] + self.buck_col[c, t] * P

        self.idx16 = np.zeros((NCORES, self.total_idx), dtype=np.int16)
        self.dstloc = np.full((NCORES, self.total_idx), -1, dtype=np.int16)
        # rank of each edge within its (core, chunk, tile) bucket
        grp_key = (core_o * nch + ch_o) * n_tiles + tl_o
        # stable sort keeps original order; compute rank via cumcount
        uniq, inv, cnt = np.unique(grp_key, return_inverse=True, return_counts=True)
        starts = np.concatenate([[0], np.cumsum(cnt)])[:-1]
        rank = np.arange(len(grp_key)) - starts[inv]
        pos = base_tc[ch_o, tl_o] + rank
        self.idx16[core_o, pos] = (src_o - ch_o * self.chunk).astype(np.int16)
        self.dstloc[core_o, pos] = dst_o.astype(np.int16)

    def wrapped_idx(self, core):
        """[128, total_idx//16] int16, wrapped-16 and replicated to 8 groups."""
        v = self.idx16[core].reshape(-1, 16).T  # [16, total/16]
        return np.tile(v, (8, 1)).copy()

    def s_blob(self, core):
        """[128, total_slots*128] fp8: per slot S[e,d] = (dstloc[e]==d).

        Slot order: tile-major (tile t: its chunk-0 slots then chunk-1 slots),
        matching the matmul loop.  Column range of tile t: s_off[t]*128.
        """
        nch = self.n_chunks
        out = np.zeros((P, self.total_slots * P), dtype=NP_F8)
        iota = np.arange(P, dtype=np.int16)
        for t in range(self.n_tiles):
            si = self.s_off[t]
            for c in range(nch):
                nsl = self.caps[t, c] // P
                if nsl == 0:
                    continue
                g = t // GROUP
                base = self.call_off[c, g] + self.buck_col[c, t] * P
                d = self.dstloc[core, base : base + nsl * P].reshape(nsl, P)
                # S [slot, e, d]
                s = (d[:, :, None] == iota[None, None, :]).astype(NP_F8)
                # [P(e), nsl, P(d)] -> columns
                out[:, si * P : (si + nsl) * P] = (
                    s.transpose(1, 0, 2).reshape(P, nsl * P)
                )
                si += nsl
        return out


def _build_program(pm, pc, n_t, s_pad, st_pad, tn, tt):
    """pm: main-pass EdgePlan (NT->NT), pc: c-pass plan (T->NT)."""
    nt_pad = NCORES * s_pad
    nc = bacc.Bacc(None, target_bir_lowering=False, num_devices=NCORES)

    def param(name, shape, dt, out=False):
        return nc.declare_dram_parameter(name, list(shape), dt, isOutput=out)

    tbl_init = param("tbl_init", (nt_pad, TPAD), F16)
    tbl_t1 = param("tbl_t1", (pc.n_chunks * pc.chunk, TPAD), F16)
    tbl_t2 = param("tbl_t2", (pc.n_chunks * pc.chunk, TPAD), F16)
    idx_nt = param("idx_nt", (P, pm.total_idx // 16), I16)
    idx_t = param("idx_t", (P, pc.total_idx // 16), I16)
    s_nt = param("s_nt", (P, pm.total_slots * P), F8)
    s_t = param("s_t", (P, pc.total_slots * P), F8)
    xnt = param("xnt", (512, s_pad), F16)  # pre-transposed on host
    xt = param("xt", (512, st_pad), F16)
    w1t = param("w1t", (512, 256), F16)
    b1 = param("b1", (256, 1), F32)
    w2t = param("w2t", (256, 40), F16)
    b2b = param("b2b", (P, 40), F32)
    alpha_nt = param("alpha_nt", (s_pad, 1), F32)
    alpha_t = param("alpha_t", (st_pad, 1), F32)
    dis_nt = param("dis_nt", (s_pad, 1), F32)
    dissq_nt = param("dissq_nt", (s_pad, 1), F32)
    hard_t = param("hard_t", (st_pad, 40), F32)
    out_nt = param("out_nt", (s_pad, 40), F32, out=True)
    out_t = param("out_t", (st_pad, 40), F32, out=True)

    table = nc.dram_tensor("table", [nt_pad, TPAD], F16)
    # ping-pong the collective in/out buffers: a lagging peer may still be
    # pulling iteration k's data after our collective instruction completed,
    # so iteration k+1 must not overwrite the same buffers
    cown = [nc.dram_tensor(f"cown{i}", [s_pad, 40], F16) for i in range(2)]
    callg = [
        nc.dram_tensor(f"callg{i}", [nt_pad, 40], F16, addr_space="Shared")
        for i in range(2)
    ]

    RG = [list(range(NCORES))]

    with tile.TileContext(nc) as tc:
        with (
            tc.tile_pool(name="persist", bufs=1) as pp,
            tc.tile_pool(name="work", bufs=4) as wp,
            tc.tile_pool(name="gpool", bufs=4) as gp,
            tc.tile_pool(name="spool", bufs=3) as sp,
            tc.tile_pool(name="mpsum", bufs=2, space="PSUM") as mp,
            tc.tile_pool(name="apsum", bufs=4, space="PSUM") as ap_,
        ):
            # one-time init: fills pad columns so later strided updates leave
            # only finite data for gathers
            nc.sync.dma_start(out=table[:, :], in_=tbl_init[:, :])

            # ---- persistent SBUF ----
            idxm_sb = pp.tile([P, pm.total_idx // 16], I16, tag="idxm")
            nc.sync.dma_start(out=idxm_sb[:], in_=idx_nt[:, :])
            idxc_sb = pp.tile([P, pc.total_idx // 16], I16, tag="idxc")
            nc.sync.dma_start(out=idxc_sb[:], in_=idx_t[:, :])

            ft_nt = pp.tile([P, tn, 40], F32, tag="ftnt")
            ft_t = pp.tile([P, tt, 40], F32, tag="ftt")
            c1 = pp.tile([P, tn, 40], F32, tag="c1")
            c2 = pp.tile([P, tn, 40], F32, tag="c2")
            compact = pp.tile([P, tn, 40], F16, tag="compact")

            w1_sb = pp.tile([P, 4, 256], F16, tag="w1")
            nc.sync.dma_start(
                out=w1_sb[:], in_=w1t.ap().rearrange("(k p) h -> p k h", p=P)
            )
            w2_sb = pp.tile([P, 2, 40], F16, tag="w2")
            nc.sync.dma_start(
                out=w2_sb[:], in_=w2t.ap().rearrange("(h p) c -> p h c", p=P)
            )
            b1_sb = pp.tile([P, 2], F32, tag="b1")
            nc.sync.dma_start(
                out=b1_sb[:], in_=b1.ap().rearrange("(h p) o -> p (h o)", p=P)
            )
            b2_sb = pp.tile([P, 40], F32, tag="b2")
            nc.sync.dma_start(out=b2_sb[:], in_=b2b[:, :])

            def cols_load(prm, n_tiles, tag):
                t_ = pp.tile([P, n_tiles], F32, tag=tag)
                nc.sync.dma_start(
                    out=t_[:], in_=prm.ap().rearrange("(t p) o -> p (t o)", p=P)
                )
                return t_

            disn_sb = cols_load(dis_nt, tn, "disn")
            dsqn_sb = cols_load(dissq_nt, tn, "dsqn")
            aln_sb = cols_load(alpha_nt, tn, "aln")
            alt_sb = cols_load(alpha_t, tt, "alt")

            # sigmoid(alpha); a*dis; 1-a
            sign_sb = pp.tile([P, tn], F32, tag="sign")
            nc.scalar.activation(
                sign_sb[:], aln_sb[:], mybir.ActivationFunctionType.Sigmoid
            )
            sigt_sb = pp.tile([P, tt], F32, tag="sigt")
            nc.scalar.activation(
                sigt_sb[:], alt_sb[:], mybir.ActivationFunctionType.Sigmoid
            )
            disa_sb = pp.tile([P, tn], F32, tag="disa")
            nc.vector.tensor_tensor(
                out=disa_sb[:], in0=sign_sb[:], in1=disn_sb[:],
                op=mybir.AluOpType.mult,
            )
            oman_sb = pp.tile([P, tn], F32, tag="oman")
            nc.vector.tensor_scalar(
                out=oman_sb[:], in0=sign_sb[:], scalar1=-1.0, scalar2=1.0,
                op0=mybir.AluOpType.mult, op1=mybir.AluOpType.add,
            )
            omat_sb = pp.tile([P, tt], F32, tag="omat")
            nc.vector.tensor_scalar(
                out=omat_sb[:], in0=sigt_sb[:], scalar1=-1.0, scalar2=1.0,
                op0=mybir.AluOpType.mult, op1=mybir.AluOpType.add,
            )

            # ---- MLP (FT branch) ----
            def mlp(xsrc, n_tiles, ft_dst):
                for n in range(n_tiles):
                    xTs = []
                    for k in range(4):
                        xT = wp.tile([P, P], F16, tag="xT")
                        nc.sync.dma_start(
                            out=xT[:],
                            in_=xsrc[k * P : (k + 1) * P, n * P : (n + 1) * P],
                        )
                        xTs.append(xT)
                    ps2 = mp.tile([P, 40], F32, tag="ps2")
                    for h in range(2):
                        ps1 = mp.tile([P, P], F32, tag="ps1")
                        for k in range(4):
                            nc.tensor.matmul(
                                ps1[:],
                                lhsT=w1_sb[:, k, h * P : (h + 1) * P],
                                rhs=xTs[k][:],
                                start=(k == 0),
                                stop=(k == 3),
                            )
                        hT = wp.tile([P, P], F16, tag="hT")
                        nc.scalar.activation(
                            hT[:], ps1[:], mybir.ActivationFunctionType.Relu,
                            bias=b1_sb[:, h : h + 1],
                        )
                        nc.tensor.matmul(
                            ps2[:], lhsT=hT[:], rhs=w2_sb[:, h, :],
                            start=(h == 0), stop=(h == 1),
                        )
                    nc.vector.tensor_tensor(
                        out=ft_dst[:, n, :], in0=ps2[:], in1=b2_sb[:],
                        op=mybir.AluOpType.add,
                    )

            mlp(xnt, tn, ft_nt)
            mlp(xt, tt, ft_t)

            # ---- generic SpMM pass ----
            _regs = {}

            def num_reg(v):
                if v not in _regs:
                    _regs[v] = nc.gpsimd.to_reg(v)
                return _regs[v]

            def spmm_pass(plan, tsrc, idx_sb, s_param, evac):
                """tsrc: DRAM table. evac(t, psum_ap) -> emits eviction."""
                nch = plan.n_chunks
                for g in range(plan.n_groups):
                    gbufs = []
                    for c in range(nch):
                        num = int(plan.call_num[c, g])
                        if num == 0:
                            gbufs.append(None)
                            continue
                        gb = gp.tile([P, num // P, TPAD], F16, tag="gb")
                        r0 = c * plan.chunk
                        nrow = plan.chunk
                        if os.environ.get("KERNEL_NO_GATHER", "0") == "1":
                            # debug: sequential read instead of gather
                            nc.sync.dma_start(
                                out=gb[:],
                                in_=tsrc[r0 : r0 + num, :].rearrange(
                                    "(n p) e -> p n e", p=P
                                ),
                            )
                        else:
                            for off, n_, col0 in plan.subcalls[(c, g)]:
                                nc.gpsimd.dma_gather(
                                    out_ap=gb[:, col0 : col0 + n_ // P, :],
                                    in_ap=tsrc[r0 : r0 + nrow, :],
                                    idxs_ap=idx_sb[:, off // 16 : (off + n_) // 16],
                                    num_idxs=n_,
                                    num_idxs_reg=num_reg(n_),
                                    elem_size=TPAD,
                                )
                        gbufs.append(gb)
                    for t in range(g * GROUP, min((g + 1) * GROUP, plan.n_tiles)):
                        tot = int(plan.slots_per_tile[t])
                        if tot == 0:
                            continue
                        si = int(plan.s_off[t])
                        st_ = sp.tile([P, tot * P], F8, tag="sstr")
                        nc.sync.dma_start(
                            out=st_[:], in_=s_param[:, si * P : (si + tot) * P]
                        )
                        ps = ap_.tile([P, 40], F32, tag="acc")
                        k = 0
                        for c in range(nch):
                            nsl = int(plan.caps[t, c]) // P
                            bc = int(plan.buck_col[c, t])
                            for j in range(nsl):
                                nc.tensor.matmul(
                                    ps[:],
                                    lhsT=st_[:, k * P : (k + 1) * P],
                                    rhs=gbufs[c][:, bc + j, 0:40],
                                    start=(k == 0),
                                    stop=(k == tot - 1),
                                )
                                k += 1
                        evac(t, ps)

            # ---- c1 / c2 passes (T sources; streamed fp8 S) ----
            def evac_c(dst):
                def f(t, ps):
                    nc.vector.tensor_copy(out=dst[:, t, :], in_=ps[:])
                return f

            spmm_pass(pc, tbl_t1, idxc_sb, s_t, evac_c(c1))
            spmm_pass(pc, tbl_t2, idxc_sb, s_t, evac_c(c2))

            # ---- 10 PLP iterations ----
            for it in range(10):
                tsrc = tbl_init if it == 0 else table
                cbuf = c1 if it == 0 else c2

                if it < 9:
                    def evac_iter(t, ps, cbuf=cbuf):
                        tmp = wp.tile([P, 40], F32, tag="ev")
                        nc.vector.tensor_tensor(
                            out=tmp[:], in0=ps[:], in1=cbuf[:, t, :],
                            op=mybir.AluOpType.add,
                        )
                        nc.vector.tensor_scalar(
                            out=compact[:, t, :], in0=tmp[:],
                            scalar1=dsqn_sb[:, t : t + 1], scalar2=None,
                            op0=mybir.AluOpType.mult,
                        )
                else:
                    def evac_iter(t, ps, cbuf=cbuf):
                        tmp = wp.tile([P, 40], F32, tag="ev")
                        nc.vector.tensor_tensor(
                            out=tmp[:], in0=ps[:], in1=cbuf[:, t, :],
                            op=mybir.AluOpType.add,
                        )
                        t2 = wp.tile([P, 40], F32, tag="ev2")
                        nc.vector.tensor_scalar(
                            out=t2[:], in0=tmp[:],
                            scalar1=disa_sb[:, t : t + 1], scalar2=None,
                            op0=mybir.AluOpType.mult,
                        )
                        t3 = wp.tile([P, 40], F32, tag="ev3")
                        nc.vector.tensor_scalar(
                            out=t3[:], in0=ft_nt[:, t, :],
                            scalar1=oman_sb[:, t : t + 1], scalar2=None,
                            op0=mybir.AluOpType.mult,
                        )
                        t4 = wp.tile([P, 40], F32, tag="ev4")
                        nc.vector.tensor_tensor(
                            out=t4[:], in0=t2[:], in1=t3[:],
                            op=mybir.AluOpType.add,
                        )
                        nc.sync.dma_start(
                            out=out_nt[t * P : (t + 1) * P, :], in_=t4[:]
                        )

                spmm_pass(pm, tsrc, idxm_sb, s_nt, evac_iter)

                if it < 9:
                    cw, cg = cown[it % 2], callg[it % 2]
                    nc.sync.dma_start(
                        out=cw.ap().rearrange("(t p) c -> p t c", p=P),
                        in_=compact[:],
                    )
                    if os.environ.get("KERNEL_NO_CC", "0") == "1":
                        # debug mode: skip the collective (wrong cross-core data)
                        nc.sync.dma_start(
                            out=cg[0 : s_pad, :], in_=cw[:, :]
                        )
                    else:
                        nc.gpsimd.collective_compute(
                            "AllGather",
                            mybir.AluOpType.bypass,
                            replica_groups=RG,
                            ins=[cw.ap().opt()],
                            outs=[cg.ap().opt()],
                        )
                    nc.sync.dma_start(out=table[:, 0:40], in_=cg[:, :])

            # ---- T-side final combine ----
            for t in range(tt):
                hsb = wp.tile([P, 40], F32, tag="hsb")
                nc.sync.dma_start(out=hsb[:], in_=hard_t[t * P : (t + 1) * P, :])
                t1_ = wp.tile([P, 40], F32, tag="tc1")
                nc.vector.tensor_scalar(
                    out=t1_[:], in0=hsb[:], scalar1=sigt_sb[:, t : t + 1],
                    scalar2=None, op0=mybir.AluOpType.mult,
                )
                t2_ = wp.tile([P, 40], F32, tag="tc2")
                nc.vector.tensor_scalar(
                    out=t2_[:], in0=ft_t[:, t, :], scalar1=omat_sb[:, t : t + 1],
                    scalar2=None, op0=mybir.AluOpType.mult,
                )
                t3_ = wp.tile([P, 40], F32, tag="tc3")
                nc.vector.tensor_tensor(
                    out=t3_[:], in0=t1_[:], in1=t2_[:],
                    op=mybir.AluOpType.add,
                )
                nc.sync.dma_start(out=out_t[t * P : (t + 1) * P, :], in_=t3_[:])

    nc.compile()
    return nc


def kernel(**inputs):
    x = np.asarray(inputs["x"], dtype=np.float32)
    edge_index = np.asarray(inputs["edge_index"])
    label_init = np.asarray(inputs["label_init"], dtype=np.float32)
    train_mask = np.asarray(inputs["train_mask"]).astype(bool)
    hard = np.asarray(inputs["hard_one_hot"], dtype=np.float32)
    fc1_w = np.asarray(inputs["fc1_w"], dtype=np.float32)
    fc1_b = np.asarray(inputs["fc1_b"], dtype=np.float32)
    fc2_w = np.asarray(inputs["fc2_w"], dtype=np.float32)
    fc2_b = np.asarray(inputs["fc2_b"], dtype=np.float32)
    alpha = np.asarray(inputs["alpha"], dtype=np.float32)

    n = x.shape[0]
    row = edge_index[0].astype(np.int64)
    col = edge_index[1].astype(np.int64)

    deg = np.bincount(row, minlength=n).astype(np.float64) + 1.0
    dis = (1.0 / np.sqrt(deg)).astype(np.float32)

    nt_ids = np.nonzero(~train_mask)[0]
    t_ids = np.nonzero(train_mask)[0]
    n_nt, n_t = len(nt_ids), len(t_ids)

    s_real = _ceil(n_nt, NCORES)
    tn = _ceil(s_real, P)
    s_pad = tn * P
    nt_pad = NCORES * s_pad
    st_real = _ceil(n_t, NCORES)
    tt = _ceil(st_real, P)
    st_pad = tt * P

    # padded NT id / compact T id for each original node
    pid = np.full(n, -1, dtype=np.int64)
    j = np.arange(n_nt)
    stripe = j // s_real
    pid[nt_ids] = stripe * s_pad + (j - stripe * s_real)
    tix = np.full(n, -1, dtype=np.int64)
    tix[t_ids] = np.arange(n_t)

    # edges into NT dsts
    sel = ~train_mask[col]
    es, ed = row[sel], col[sel]
    src_nt = ~train_mask[es]
    # main: NT->NT plus self-loops on NT
    m_src = np.concatenate([pid[es[src_nt]], pid[nt_ids]])
    m_dst = np.concatenate([pid[ed[src_nt]], pid[nt_ids]])
    pm = EdgePlan(m_src, m_dst, nt_pad, s_pad, tn)
    # cpass: T->NT
    c_src = tix[es[~src_nt]]
    c_dst = pid[ed[~src_nt]]
    pc = EdgePlan(c_src, c_dst, n_t, s_pad, tn)

    # ---- tables ----
    scaled_li = dis[:, None] * label_init  # [n, 40]
    scaled_hd = dis[:, None] * hard

    def pack_rows(rows40):
        out = np.zeros((rows40.shape[0], TPAD), dtype=np.float16)
        out[:, :40] = rows40.astype(np.float16)
        return out

    tbl_init_g = np.zeros((nt_pad, TPAD), dtype=np.float16)
    tbl_init_g[pid[nt_ids], :40] = scaled_li[nt_ids].astype(np.float16)
    t_rows = pc.n_chunks * pc.chunk
    tbl_t1_g = np.zeros((t_rows, TPAD), dtype=np.float16)
    tbl_t1_g[: n_t, :40] = scaled_li[t_ids].astype(np.float16)
    tbl_t2_g = np.zeros((t_rows, TPAD), dtype=np.float16)
    tbl_t2_g[: n_t, :40] = scaled_hd[t_ids].astype(np.float16)

    # ---- per-core MLP / combine inputs ----
    def stripe_rows(ids, srl, spad_, nstripes=NCORES):
        """Return [nstripes, spad_] original-id per padded slot (-1 pad)."""
        m = np.full((nstripes, spad_), -1, dtype=np.int64)
        for i in range(nstripes):
            lo = i * srl
            hi = min(len(ids), (i + 1) * srl)
            if hi > lo:
                m[i, : hi - lo] = ids[lo:hi]
        return m

    nt_map = stripe_rows(nt_ids, s_real, s_pad)
    t_map = stripe_rows(t_ids, st_real, st_pad)

    def take(arr, idmap, fill=0.0):
        out = np.full((idmap.shape[0], idmap.shape[1]) + arr.shape[1:], fill,
                      dtype=arr.dtype)
        valid = idmap >= 0
        out[valid] = arr[idmap[valid]]
        return out

    xnt_g = np.ascontiguousarray(
        take(x, nt_map).astype(np.float16).transpose(0, 2, 1)
    )
    xt_g = np.ascontiguousarray(take(x, t_map).astype(np.float16).transpose(0, 2, 1))
    al_nt_g = take(alpha, nt_map).astype(np.float32)
    al_t_g = take(alpha, t_map).astype(np.float32)
    dis_nt_g = take(dis[:, None], nt_map).astype(np.float32)
    dsq_nt_g = take((dis * dis)[:, None], nt_map).astype(np.float32)
    hard_t_g = take(hard, t_map).astype(np.float32)

    w1t_g = fc1_w.T.astype(np.float16).copy()  # [512, 256]
    b1_g = fc1_b.reshape(256, 1).astype(np.float32)
    w2t_g = fc2_w.T.astype(np.float16).copy()  # [256, 40]
    b2b_g = np.tile(fc2_b.reshape(1, 40), (P, 1)).astype(np.float32)

    nc = _build_program(pm, pc, n_t, s_pad, st_pad, tn, tt)

    if os.environ.get("KERNEL_BUILD_ONLY", "0") == "1":
        e = BuildOnly()
        e.nc = nc
        raise e

    in_maps = []
    for i in range(NCORES):
        in_maps.append(
            dict(
                tbl_init=tbl_init_g,
                tbl_t1=tbl_t1_g,
                tbl_t2=tbl_t2_g,
                idx_nt=pm.wrapped_idx(i),
                idx_t=pc.wrapped_idx(i),
                s_nt=pm.s_blob(i),
                s_t=pc.s_blob(i),
                xnt=xnt_g[i],
                xt=xt_g[i],
                w1t=w1t_g,
                b1=b1_g,
                w2t=w2t_g,
                b2b=b2b_g,
                alpha_nt=al_nt_g[i],
                alpha_t=al_t_g[i],
                dis_nt=dis_nt_g[i],
                dissq_nt=dsq_nt_g[i],
                hard_t=hard_t_g[i],
            )
        )

    if os.environ.get("KERNEL_SIM", "0") == "1":
        from concourse import bass_interp

        sim = bass_interp.MultiCoreSim(nc, NCORES)
        for i in range(NCORES):
            for k, v in in_maps[i].items():
                sim.cores[i].tensor(k)[:] = v
        sim.simulate()
        results = [
            {k: np.array(sim.cores[i].mem_tensor(k)) for k in ("out_nt", "out_t")}
            for i in range(NCORES)
        ]
        res = None
    else:
        res = run_bass_kernel_spmd(
            nc, in_maps, core_ids=list(range(NCORES)),
            trace=bool(int(os.environ.get("KERNEL_TRACE", "0"))),
        )
        results = res.results
        nbench = int(os.environ.get("KERNEL_BENCH", "0"))
        if nbench > 0:
            import time as _time

            times = []
            for _ in range(nbench):
                t0 = _time.time()
                run_bass_kernel_spmd(nc, in_maps, core_ids=list(range(NCORES)))
                times.append(_time.time() - t0)
            kernel.last_bench_s = min(times)
    kernel.last_results = res
    kernel.last_nc = nc
    kernel.last_in_maps = in_maps

    out = np.zeros((n, 40), dtype=np.float32)
    for i in range(NCORES):
        om = results[i]["out_nt"]
        ot = results[i]["out_t"]
        v = nt_map[i] >= 0
        out[nt_map[i][v]] = om[v]
        v = t_map[i] >= 0
        out[t_map[i][v]] = ot[v]
    return out



# revision 25
# speedup vs baseline: 5.4025x; 1.4616x over previous
"""CPFStudent (GNN label propagation + MLP mix) on 8 TRN2 NeuronCores.

Strategy (dst-sharded SpMM with selector matmuls):
  - Reference: 10 PLP steps of plp <- where(mask, hard, A_hat @ plp), with
    A_hat = D^-1/2 (A+I) D^-1/2 built from out-degrees of edge_index[0];
    final logits = sigmoid(alpha)*plp + (1-sigmoid(alpha))*relu(x@W1^T+b1)@W2^T+b2.
  - Only non-train (NT) rows of plp evolve; train (T) rows are constant after
    step 1.  We keep the state as table = dis * plp (dis = deg^-1/2), fp16,
    so per-edge messages need no norm multiply:
        plp_new[d] = dis[d] * ( sum_{e: src NT} table[src] + c )
    where c is a constant per dst: c1 (from dis*label_init over T srcs, used in
    step 1) or c2 (from dis*hard over T srcs, steps 2..10).
  - Nodes are permuted host-side: NT nodes first, padded per-core stripes.
    Each core owns a contiguous stripe of NT dst rows; edges are bucketed by
    (dst_tile of 128, src chunk of <=32768 rows) host-side, padded to uniform
    capacities across cores (SPMD), and gathered per iteration with
    gpsimd.dma_gather (256B elements) from an HBM fp16 table.
  - Scatter/segment-sum is done on the TensorEngine: per 128-edge slot a
    host-precomputed fp8 selector S (S[e,d]=1 iff dst_local(e)==d) multiplies
    the gathered messages, accumulating in PSUM per dst tile.
  - Per-iteration halo exchange: AllGather of each core's new compact fp16
    rows, then a strided DMA expands them into the 256B-strided table.
"""

import math
import os
import sys

import numpy as np

sys.path.insert(0, "/opt/trn_rl_repo")

import ml_dtypes  # noqa: E402

import concourse.bass as bass  # noqa: E402
import concourse.mybir as mybir  # noqa: E402
import concourse.tile as tile  # noqa: E402
from concourse import bacc  # noqa: E402
from concourse.bass_utils import run_bass_kernel_spmd  # noqa: E402

P = 128
NCORES = 8
TPAD = 128  # fp16 elements per table row (256B, dma_gather elem granularity)
GROUP = 7  # dst tiles per dma_gather call group
MAX_CALL = int(os.environ.get("KERNEL_MAX_CALL", "1024"))
NQUEUES = int(os.environ.get("KERNEL_QUEUES", "1"))

F16 = mybir.dt.float16
F32 = mybir.dt.float32
F8 = mybir.dt.float8e4
I16 = mybir.dt.int16
NP_F8 = ml_dtypes.float8_e4m3


def _ceil(a, b):
    return -(-a // b)


class BuildOnly(Exception):
    pass


class EdgePlan:
    """Host-side bucketed edge plan for one SpMM pass, uniform across cores.

    src_row: int array, row index into the pass's gather table
    dst_pid: int array, padded NT id of the destination
    """

    def __init__(self, src_row, dst_pid, n_rows, s_pad, n_tiles):
        self.n_chunks = max(1, _ceil(n_rows, 32768))
        self.chunk = _ceil(n_rows, self.n_chunks)
        self.n_tiles = n_tiles
        nch = self.n_chunks

        core = dst_pid // s_pad
        dloc = dst_pid - core * s_pad
        tl = dloc // P
        dstloc = dloc % P
        ch = src_row // self.chunk

        key = (core * n_tiles + tl) * nch + ch
        counts = np.bincount(key, minlength=NCORES * n_tiles * nch).reshape(
            NCORES, n_tiles, nch
        )
        caps = counts.max(axis=0)  # [n_tiles, nch]
        caps = ((caps + P - 1) // P) * P
        self.caps = caps
        self.slots_per_tile = caps.sum(axis=1) // P  # [n_tiles]
        self.s_off = np.concatenate([[0], np.cumsum(self.slots_per_tile)])
        self.total_slots = int(self.s_off[-1])

        # per (chunk, group) call: num idxs and per-tile column offsets
        self.n_groups = _ceil(n_tiles, GROUP)
        self.call_num = np.zeros((nch, self.n_groups), dtype=np.int64)
        self.buck_col = np.zeros((nch, n_tiles), dtype=np.int64)  # col in its call buf
        for c in range(nch):
            for g in range(self.n_groups):
                off = 0
                for t in range(g * GROUP, min((g + 1) * GROUP, n_tiles)):
                    self.buck_col[c, t] = off
                    off += caps[t, c] // P
                self.call_num[c, g] = off * P
        # col offset of each call inside the flat idx stream (per chunk then group)
        self.call_off = np.zeros((nch, self.n_groups), dtype=np.int64)
        off = 0
        for c in range(nch):
            for g in range(self.n_groups):
                self.call_off[c, g] = off
                off += self.call_num[c, g]
        self.total_idx = off

        # sub-calls of <= MAX_CALL idxs: per (c, g) a list of (idx_off, num, col0)
        self.subcalls = {}
        for c in range(nch):
            for g in range(self.n_groups):
                num = int(self.call_num[c, g])
                base = int(self.call_off[c, g])
                subs = []
                p0 = 0
                while p0 < num:
                    n_ = min(MAX_CALL, num - p0)
                    subs.append((base + p0, n_, p0 // P))
                    p0 += n_
                self.subcalls[(c, g)] = subs

        # order edges by (core, chunk, tile); build padded per-core streams
        order = np.argsort((core * nch + ch) * n_tiles + tl, kind="stable")
        src_o = src_row[order]
        core_o = core[order]
        ch_o = ch[order]
        tl_o = tl[order]
        dst_o = dstloc[order]

        # destination position of each edge in the padded stream
        # padded stream order: for chunk c, group g, tile t in g: cap[t,c] entries
        base_tc = np.zeros((nch, n_tiles), dtype=np.int64)
        for c in range(nch):
            for g in range(self.n_groups):
                for t in range(g * GROUP, min((g + 1) * GROUP, n_tiles)):
                    base_tc[c, t] = self.call_off[c, g] + self.buck_col[c, t] * P

        self.idx16 = np.zeros((NCORES, self.total_idx), dtype=np.int16)
        self.dstloc = np.full((NCORES, self.total_idx), -1, dtype=np.int16)
        # rank of each edge within its (core, chunk, tile) bucket
        grp_key = (core_o * nch + ch_o) * n_tiles + tl_o
        # stable sort keeps original order; compute rank via cumcount
        uniq, inv, cnt = np.unique(grp_key, return_inverse=True, return_counts=True)
        starts = np.concatenate([[0], np.cumsum(cnt)])[:-1]
        rank = np.arange(len(grp_key)) - starts[inv]
        pos = base_tc[ch_o, tl_o] + rank
        self.idx16[core_o, pos] = (src_o - ch_o * self.chunk).astype(np.int16)
        self.dstloc[core_o, pos] = dst_o.astype(np.int16)

    def wrapped_idx(self, core):
        """[128, total_idx//16] int16, wrapped-16 and replicated to 8 groups."""
        v = self.idx16[core].reshape(-1, 16).T  # [16, total/16]
        return np.tile(v, (8, 1)).copy()

    def s_blob(self, core):
        """[128, total_slots*128] fp8: per slot S[e,d] = (dstloc[e]==d).

        Slot order: tile-major (tile t: its chunk-0 slots then chunk-1 slots),
        matching the matmul loop.  Column range of tile t: s_off[t]*128.
        """
        nch = self.n_chunks
        out = np.zeros((P, self.total_slots * P), dtype=NP_F8)
        iota = np.arange(P, dtype=np.int16)
        for t in range(self.n_tiles):
            si = self.s_off[t]
            for c in range(nch):
                nsl = self.caps[t, c] // P
                if nsl == 0:
                    continue
                g = t // GROUP
                base = self.call_off[c, g] + self.buck_col[c, t] * P
                d = self.dstloc[core, base : base + nsl * P].reshape(nsl, P)
                # S [slot, e, d]
                s = (d[:, :, None] == iota[None, None, :]).astype(NP_F8)
                # [P(e), nsl, P(d)] -> columns
                out[:, si * P : (si + nsl) * P] = (
                    s.transpose(1, 0, 2).reshape(P, nsl * P)
                )
                si += nsl
        return out


def _build_program(pm, pc, n_t, s_pad, st_pad, tn, tt):
    """pm: main-pass EdgePlan (NT->NT), pc: c-pass plan (T->NT)."""
    nt_pad = NCORES * s_pad
    nc = bacc.Bacc(
        None,
        target_bir_lowering=False,
        num_devices=NCORES,
        num_swdge_queues=NQUEUES,
    )

    def param(name, shape, dt, out=False):
        return nc.declare_dram_parameter(name, list(shape), dt, isOutput=out)

    tbl_init = param("tbl_init", (nt_pad, TPAD), F16)
    tbl_t1 = param("tbl_t1", (pc.n_chunks * pc.chunk, TPAD), F16)
    tbl_t2 = param("tbl_t2", (pc.n_chunks * pc.chunk, TPAD), F16)
    idx_nt = param("idx_nt", (P, pm.total_idx // 16), I16)
    idx_t = param("idx_t", (P, pc.total_idx // 16), I16)
    s_nt = param("s_nt", (P, pm.total_slots * P), F8)
    s_t = param("s_t", (P, pc.total_slots * P), F8)
    xnt = param("xnt", (512, s_pad), F16)  # pre-transposed on host
    xt = param("xt", (512, st_pad), F16)
    w1t = param("w1t", (512, 256), F16)
    b1 = param("b1", (256, 1), F32)
    w2t = param("w2t", (256, 40), F16)
    b2b = param("b2b", (P, 40), F32)
    alpha_nt = param("alpha_nt", (s_pad, 1), F32)
    alpha_t = param("alpha_t", (st_pad, 1), F32)
    dis_nt = param("dis_nt", (s_pad, 1), F32)
    dissq_nt = param("dissq_nt", (s_pad, 1), F32)
    hard_t = param("hard_t", (st_pad, 40), F32)
    out_nt = param("out_nt", (s_pad, 40), F32, out=True)
    out_t = param("out_t", (st_pad, 40), F32, out=True)

    table = nc.dram_tensor("table", [nt_pad, TPAD], F16)
    # ping-pong the collective in/out buffers: a lagging peer may still be
    # pulling iteration k's data after our collective instruction completed,
    # so iteration k+1 must not overwrite the same buffers
    cown = [nc.dram_tensor(f"cown{i}", [s_pad, 40], F16) for i in range(2)]
    callg = [
        nc.dram_tensor(f"callg{i}", [nt_pad, 40], F16, addr_space="Shared")
        for i in range(2)
    ]

    RG = [list(range(NCORES))]

    with tile.TileContext(nc) as tc:
        with (
            tc.tile_pool(name="persist", bufs=1) as pp,
            tc.tile_pool(name="work", bufs=4) as wp,
            tc.tile_pool(name="gpool", bufs=4) as gp,
            tc.tile_pool(name="spool", bufs=3) as sp,
            tc.tile_pool(name="mpsum", bufs=2, space="PSUM") as mp,
            tc.tile_pool(name="apsum", bufs=4, space="PSUM") as ap_,
        ):
            # one-time init: fills pad columns so later strided updates leave
            # only finite data for gathers
            nc.sync.dma_start(out=table[:, :], in_=tbl_init[:, :])

            # ---- persistent SBUF ----
            idxm_sb = pp.tile([P, pm.total_idx // 16], I16, tag="idxm")
            nc.sync.dma_start(out=idxm_sb[:], in_=idx_nt[:, :])
            idxc_sb = pp.tile([P, pc.total_idx // 16], I16, tag="idxc")
            nc.sync.dma_start(out=idxc_sb[:], in_=idx_t[:, :])

            ft_nt = pp.tile([P, tn, 40], F32, tag="ftnt")
            ft_t = pp.tile([P, tt, 40], F32, tag="ftt")
            c1 = pp.tile([P, tn, 40], F32, tag="c1")
            c2 = pp.tile([P, tn, 40], F32, tag="c2")
            compact = pp.tile([P, tn, 40], F16, tag="compact")

            w1_sb = pp.tile([P, 4, 256], F16, tag="w1")
            nc.sync.dma_start(
                out=w1_sb[:], in_=w1t.ap().rearrange("(k p) h -> p k h", p=P)
            )
            w2_sb = pp.tile([P, 2, 40], F16, tag="w2")
            nc.sync.dma_start(
                out=w2_sb[:], in_=w2t.ap().rearrange("(h p) c -> p h c", p=P)
            )
            b1_sb = pp.tile([P, 2], F32, tag="b1")
            nc.sync.dma_start(
                out=b1_sb[:], in_=b1.ap().rearrange("(h p) o -> p (h o)", p=P)
            )
            b2_sb = pp.tile([P, 40], F32, tag="b2")
            nc.sync.dma_start(out=b2_sb[:], in_=b2b[:, :])

            def cols_load(prm, n_tiles, tag):
                t_ = pp.tile([P, n_tiles], F32, tag=tag)
                nc.sync.dma_start(
                    out=t_[:], in_=prm.ap().rearrange("(t p) o -> p (t o)", p=P)
                )
                return t_

            disn_sb = cols_load(dis_nt, tn, "disn")
            dsqn_sb = cols_load(dissq_nt, tn, "dsqn")
            aln_sb = cols_load(alpha_nt, tn, "aln")
            alt_sb = cols_load(alpha_t, tt, "alt")

            # sigmoid(alpha); a*dis; 1-a
            sign_sb = pp.tile([P, tn], F32, tag="sign")
            nc.scalar.activation(
                sign_sb[:], aln_sb[:], mybir.ActivationFunctionType.Sigmoid
            )
            sigt_sb = pp.tile([P, tt], F32, tag="sigt")
            nc.scalar.activation(
                sigt_sb[:], alt_sb[:], mybir.ActivationFunctionType.Sigmoid
            )
            disa_sb = pp.tile([P, tn], F32, tag="disa")
            nc.vector.tensor_tensor(
                out=disa_sb[:], in0=sign_sb[:], in1=disn_sb[:],
                op=mybir.AluOpType.mult,
            )
            oman_sb = pp.tile([P, tn], F32, tag="oman")
            nc.vector.tensor_scalar(
                out=oman_sb[:], in0=sign_sb[:], scalar1=-1.0, scalar2=1.0,
                op0=mybir.AluOpType.mult, op1=mybir.AluOpType.add,
            )
            omat_sb = pp.tile([P, tt], F32, tag="omat")
            nc.vector.tensor_scalar(
                out=omat_sb[:], in0=sigt_sb[:], scalar1=-1.0, scalar2=1.0,
                op0=mybir.AluOpType.mult, op1=mybir.AluOpType.add,
            )

            # ---- MLP (FT branch) ----
            def mlp(xsrc, n_tiles, ft_dst):
                for n in range(n_tiles):
                    xTs = []
                    for k in range(4):
                        xT = wp.tile([P, P], F16, tag="xT")
                        nc.sync.dma_start(
                            out=xT[:],
                            in_=xsrc[k * P : (k + 1) * P, n * P : (n + 1) * P],
                        )
                        xTs.append(xT)
                    ps2 = mp.tile([P, 40], F32, tag="ps2")
                    for h in range(2):
                        ps1 = mp.tile([P, P], F32, tag="ps1")
                        for k in range(4):
                            nc.tensor.matmul(
                                ps1[:],
                                lhsT=w1_sb[:, k, h * P : (h + 1) * P],
                                rhs=xTs[k][:],
                                start=(k == 0),
                                stop=(k == 3),
                            )
                        hT = wp.tile([P, P], F16, tag="hT")
                        nc.scalar.activation(
                            hT[:], ps1[:], mybir.ActivationFunctionType.Relu,
                            bias=b1_sb[:, h : h + 1],
                        )
                        nc.tensor.matmul(
                            ps2[:], lhsT=hT[:], rhs=w2_sb[:, h, :],
                            start=(h == 0), stop=(h == 1),
                        )
                    nc.vector.tensor_tensor(
                        out=ft_dst[:, n, :], in0=ps2[:], in1=b2_sb[:],
                        op=mybir.AluOpType.add,
                    )

            mlp(xnt, tn, ft_nt)
            mlp(xt, tt, ft_t)

            # ---- generic SpMM pass ----
            _regs = {}

            def num_reg(v):
                if v not in _regs:
                    _regs[v] = nc.gpsimd.to_reg(v)
                return _regs[v]

            def spmm_pass(plan, tsrc, idx_sb, s_param, evac):
                """tsrc: DRAM table. evac(t, psum_ap) -> emits eviction."""
                nch = plan.n_chunks
                for g in range(plan.n_groups):
                    gbufs = []
                    for c in range(nch):
                        num = int(plan.call_num[c, g])
                        if num == 0:
                            gbufs.append(None)
                            continue
                        gb = gp.tile([P, num // P, TPAD], F16, tag="gb")
                        r0 = c * plan.chunk
                        nrow = plan.chunk
                        if os.environ.get("KERNEL_NO_GATHER", "0") == "1":
                            # debug: sequential read instead of gather
                            nc.sync.dma_start(
                                out=gb[:],
                                in_=tsrc[r0 : r0 + num, :].rearrange(
                                    "(n p) e -> p n e", p=P
                                ),
                            )
                        else:
                            for off, n_, col0 in plan.subcalls[(c, g)]:
                                nc.gpsimd.dma_gather(
                                    out_ap=gb[:, col0 : col0 + n_ // P, :],
                                    in_ap=tsrc[r0 : r0 + nrow, :],
                                    idxs_ap=idx_sb[:, off // 16 : (off + n_) // 16],
                                    num_idxs=n_,
                                    num_idxs_reg=num_reg(n_),
                                    elem_size=TPAD,
                                    queue_num=spmm_pass.qi % NQUEUES,
                                )
                                spmm_pass.qi += 1
                        gbufs.append(gb)
                    for t in range(g * GROUP, min((g + 1) * GROUP, plan.n_tiles)):
                        tot = int(plan.slots_per_tile[t])
                        if tot == 0:
                            continue
                        si = int(plan.s_off[t])
                        st_ = sp.tile([P, tot * P], F8, tag="sstr")
                        nc.sync.dma_start(
                            out=st_[:], in_=s_param[:, si * P : (si + tot) * P]
                        )
                        ps = ap_.tile([P, 40], F32, tag="acc")
                        k = 0
                        for c in range(nch):
                            nsl = int(plan.caps[t, c]) // P
                            bc = int(plan.buck_col[c, t])
                            for j in range(nsl):
                                nc.tensor.matmul(
                                    ps[:],
                                    lhsT=st_[:, k * P : (k + 1) * P],
                                    rhs=gbufs[c][:, bc + j, 0:40],
                                    start=(k == 0),
                                    stop=(k == tot - 1),
                                )
                                k += 1
                        evac(t, ps)

            # ---- c1 / c2 passes (T sources; streamed fp8 S) ----
            def evac_c(dst):
                def f(t, ps):
                    nc.vector.tensor_copy(out=dst[:, t, :], in_=ps[:])
                return f

            spmm_pass.qi = 0
            spmm_pass(pc, tbl_t1, idxc_sb, s_t, evac_c(c1))
            spmm_pass(pc, tbl_t2, idxc_sb, s_t, evac_c(c2))

            # ---- 10 PLP iterations ----
            for it in range(10):
                tsrc = tbl_init if it == 0 else table
                cbuf = c1 if it == 0 else c2

                if it < 9:
                    def evac_iter(t, ps, cbuf=cbuf):
                        tmp = wp.tile([P, 40], F32, tag="ev")
                        nc.vector.tensor_tensor(
                            out=tmp[:], in0=ps[:], in1=cbuf[:, t, :],
                            op=mybir.AluOpType.add,
                        )
                        nc.vector.tensor_scalar(
                            out=compact[:, t, :], in0=tmp[:],
                            scalar1=dsqn_sb[:, t : t + 1], scalar2=None,
                            op0=mybir.AluOpType.mult,
                        )
                else:
                    def evac_iter(t, ps, cbuf=cbuf):
                        tmp = wp.tile([P, 40], F32, tag="ev")
                        nc.vector.tensor_tensor(
                            out=tmp[:], in0=ps[:], in1=cbuf[:, t, :],
                            op=mybir.AluOpType.add,
                        )
                        t2 = wp.tile([P, 40], F32, tag="ev2")
                        nc.vector.tensor_scalar(
                            out=t2[:], in0=tmp[:],
                            scalar1=disa_sb[:, t : t + 1], scalar2=None,
                            op0=mybir.AluOpType.mult,
                        )
                        t3 = wp.tile([P, 40], F32, tag="ev3")
                        nc.vector.tensor_scalar(
                            out=t3[:], in0=ft_nt[:, t, :],
                            scalar1=oman_sb[:, t : t + 1], scalar2=None,
                            op0=mybir.AluOpType.mult,
                        )
                        t4 = wp.tile([P, 40], F32, tag="ev4")
                        nc.vector.tensor_tensor(
                            out=t4[:], in0=t2[:], in1=t3[:],
                            op=mybir.AluOpType.add,
                        )
                        nc.sync.dma_start(
                            out=out_nt[t * P : (t + 1) * P, :], in_=t4[:]
                        )

                spmm_pass(pm, tsrc, idxm_sb, s_nt, evac_iter)

                if it < 9:
                    cw, cg = cown[it % 2], callg[it % 2]
                    nc.sync.dma_start(
                        out=cw.ap().rearrange("(t p) c -> p t c", p=P),
                        in_=compact[:],
                    )
                    if os.environ.get("KERNEL_NO_CC", "0") == "1":
                        # debug mode: skip the collective (wrong cross-core data)
                        nc.sync.dma_start(
                            out=cg[0 : s_pad, :], in_=cw[:, :]
                        )
                    else:
                        nc.gpsimd.collective_compute(
                            "AllGather",
                            mybir.AluOpType.bypass,
                            replica_groups=RG,
                            ins=[cw.ap().opt()],
                            outs=[cg.ap().opt()],
                        )
                    half = nt_pad // 2
                    nc.sync.dma_start(
                        out=table[0:half, 0:40], in_=cg[0:half, :]
                    )
                    nc.sync.dma_start(
                        out=table[half:nt_pad, 0:40], in_=cg[half:nt_pad, :]
                    )

            # ---- T-side final combine ----
            for t in range(tt):
                hsb = wp.tile([P, 40], F32, tag="hsb")
                nc.sync.dma_start(out=hsb[:], in_=hard_t[t * P : (t + 1) * P, :])
                t1_ = wp.tile([P, 40], F32, tag="tc1")
                nc.vector.tensor_scalar(
                    out=t1_[:], in0=hsb[:], scalar1=sigt_sb[:, t : t + 1],
                    scalar2=None, op0=mybir.AluOpType.mult,
                )
                t2_ = wp.tile([P, 40], F32, tag="tc2")
                nc.vector.tensor_scalar(
                    out=t2_[:], in0=ft_t[:, t, :], scalar1=omat_sb[:, t : t + 1],
                    scalar2=None, op0=mybir.AluOpType.mult,
                )
                t3_ = wp.tile([P, 40], F32, tag="tc3")
                nc.vector.tensor_tensor(
                    out=t3_[:], in0=t1_[:], in1=t2_[:],
                    op=mybir.AluOpType.add,
                )
                nc.sync.dma_start(out=out_t[t * P : (t + 1) * P, :], in_=t3_[:])

    nc.compile()
    return nc


def kernel(**inputs):
    x = np.asarray(inputs["x"], dtype=np.float32)
    edge_index = np.asarray(inputs["edge_index"])
    label_init = np.asarray(inputs["label_init"], dtype=np.float32)
    train_mask = np.asarray(inputs["train_mask"]).astype(bool)
    hard = np.asarray(inputs["hard_one_hot"], dtype=np.float32)
    fc1_w = np.asarray(inputs["fc1_w"], dtype=np.float32)
    fc1_b = np.asarray(inputs["fc1_b"], dtype=np.float32)
    fc2_w = np.asarray(inputs["fc2_w"], dtype=np.float32)
    fc2_b = np.asarray(inputs["fc2_b"], dtype=np.float32)
    alpha = np.asarray(inputs["alpha"], dtype=np.float32)

    n = x.shape[0]
    row = edge_index[0].astype(np.int64)
    col = edge_index[1].astype(np.int64)

    deg = np.bincount(row, minlength=n).astype(np.float64) + 1.0
    dis = (1.0 / np.sqrt(deg)).astype(np.float32)

    nt_ids = np.nonzero(~train_mask)[0]
    t_ids = np.nonzero(train_mask)[0]
    n_nt, n_t = len(nt_ids), len(t_ids)

    s_real = _ceil(n_nt, NCORES)
    tn = _ceil(s_real, P)
    s_pad = tn * P
    nt_pad = NCORES * s_pad
    st_real = _ceil(n_t, NCORES)
    tt = _ceil(st_real, P)
    st_pad = tt * P

    # padded NT id / compact T id for each original node
    pid = np.full(n, -1, dtype=np.int64)
    j = np.arange(n_nt)
    stripe = j // s_real
    pid[nt_ids] = stripe * s_pad + (j - stripe * s_real)
    tix = np.full(n, -1, dtype=np.int64)
    tix[t_ids] = np.arange(n_t)

    # edges into NT dsts
    sel = ~train_mask[col]
    es, ed = row[sel], col[sel]
    src_nt = ~train_mask[es]
    # main: NT->NT plus self-loops on NT
    m_src = np.concatenate([pid[es[src_nt]], pid[nt_ids]])
    m_dst = np.concatenate([pid[ed[src_nt]], pid[nt_ids]])
    pm = EdgePlan(m_src, m_dst, nt_pad, s_pad, tn)
    # cpass: T->NT
    c_src = tix[es[~src_nt]]
    c_dst = pid[ed[~src_nt]]
    pc = EdgePlan(c_src, c_dst, n_t, s_pad, tn)

    # ---- tables ----
    scaled_li = dis[:, None] * label_init  # [n, 40]
    scaled_hd = dis[:, None] * hard

    def pack_rows(rows40):
        out = np.zeros((rows40.shape[0], TPAD), dtype=np.float16)
        out[:, :40] = rows40.astype(np.float16)
        return out

    tbl_init_g = np.zeros((nt_pad, TPAD), dtype=np.float16)
    tbl_init_g[pid[nt_ids], :40] = scaled_li[nt_ids].astype(np.float16)
    t_rows = pc.n_chunks * pc.chunk
    tbl_t1_g = np.zeros((t_rows, TPAD), dtype=np.float16)
    tbl_t1_g[: n_t, :40] = scaled_li[t_ids].astype(np.float16)
    tbl_t2_g = np.zeros((t_rows, TPAD), dtype=np.float16)
    tbl_t2_g[: n_t, :40] = scaled_hd[t_ids].astype(np.float16)

    # ---- per-core MLP / combine inputs ----
    def stripe_rows(ids, srl, spad_, nstripes=NCORES):
        """Return [nstripes, spad_] original-id per padded slot (-1 pad)."""
        m = np.full((nstripes, spad_), -1, dtype=np.int64)
        for i in range(nstripes):
            lo = i * srl
            hi = min(len(ids), (i + 1) * srl)
            if hi > lo:
                m[i, : hi - lo] = ids[lo:hi]
        return m

    nt_map = stripe_rows(nt_ids, s_real, s_pad)
    t_map = stripe_rows(t_ids, st_real, st_pad)

    def take(arr, idmap, fill=0.0):
        out = np.full((idmap.shape[0], idmap.shape[1]) + arr.shape[1:], fill,
                      dtype=arr.dtype)
        valid = idmap >= 0
        out[valid] = arr[idmap[valid]]
        return out

    xnt_g = np.ascontiguousarray(
        take(x, nt_map).astype(np.float16).transpose(0, 2, 1)
    )
    xt_g = np.ascontiguousarray(take(x, t_map).astype(np.float16).transpose(0, 2, 1))
    al_nt_g = take(alpha, nt_map).astype(np.float32)
    al_t_g = take(alpha, t_map).astype(np.float32)
    dis_nt_g = take(dis[:, None], nt_map).astype(np.float32)
    dsq_nt_g = take((dis * dis)[:, None], nt_map).astype(np.float32)
    hard_t_g = take(hard, t_map).astype(np.float32)

    w1t_g = fc1_w.T.astype(np.float16).copy()  # [512, 256]
    b1_g = fc1_b.reshape(256, 1).astype(np.float32)
    w2t_g = fc2_w.T.astype(np.float16).copy()  # [256, 40]
    b2b_g = np.tile(fc2_b.reshape(1, 40), (P, 1)).astype(np.float32)

    nc = _build_program(pm, pc, n_t, s_pad, st_pad, tn, tt)

    if os.environ.get("KERNEL_BUILD_ONLY", "0") == "1":
        e = BuildOnly()
        e.nc = nc
        raise e

    in_maps = []
    for i in range(NCORES):
        in_maps.append(
            dict(
                tbl_init=tbl_init_g,
                tbl_t1=tbl_t1_g,
                tbl_t2=tbl_t2_g,
                idx_nt=pm.wrapped_idx(i),
                idx_t=pc.wrapped_idx(i),
                s_nt=pm.s_blob(i),
                s_t=pc.s_blob(i),
                xnt=xnt_g[i],
                xt=xt_g[i],
                w1t=w1t_g,
                b1=b1_g,
                w2t=w2t_g,
                b2b=b2b_g,
                alpha_nt=al_nt_g[i],
                alpha_t=al_t_g[i],
                dis_nt=dis_nt_g[i],
                dissq_nt=dsq_nt_g[i],
                hard_t=hard_t_g[i],
            )
        )

    if os.environ.get("KERNEL_SIM", "0") == "1":
        from concourse import bass_interp

        sim = bass_interp.MultiCoreSim(nc, NCORES)
        for i in range(NCORES):
            for k, v in in_maps[i].items():
                sim.cores[i].tensor(k)[:] = v
        sim.simulate()
        results = [
            {k: np.array(sim.cores[i].mem_tensor(k)) for k in ("out_nt", "out_t")}
            for i in range(NCORES)
        ]
        res = None
    else:
        res = run_bass_kernel_spmd(
            nc, in_maps, core_ids=list(range(NCORES)),
            trace=bool(int(os.environ.get("KERNEL_TRACE", "0"))),
        )
        results = res.results
        nbench = int(os.environ.get("KERNEL_BENCH", "0"))
        if nbench > 0:
            import time as _time

            times = []
            for _ in range(nbench):
                t0 = _time.time()
                run_bass_kernel_spmd(nc, in_maps, core_ids=list(range(NCORES)))
                times.append(_time.time() - t0)
            kernel.last_bench_s = min(times)
    kernel.last_results = res
    kernel.last_nc = nc
    kernel.last_in_maps = in_maps

    out = np.zeros((n, 40), dtype=np.float32)
    for i in range(NCORES):
        om = results[i]["out_nt"]
        ot = results[i]["out_t"]
        v = nt_map[i] >= 0
        out[nt_map[i][v]] = om[v]
        v = t_map[i] >= 0
        out[t_map[i][v]] = ot[v]
    return out



# revision 26
# speedup vs baseline: 6.3853x; 1.1819x over previous
"""CPFStudent (GNN label propagation + MLP mix) on 8 TRN2 NeuronCores.

Strategy (dst-sharded SpMM with selector matmuls):
  - Reference: 10 PLP steps of plp <- where(mask, hard, A_hat @ plp), with
    A_hat = D^-1/2 (A+I) D^-1/2 built from out-degrees of edge_index[0];
    final logits = sigmoid(alpha)*plp + (1-sigmoid(alpha))*relu(x@W1^T+b1)@W2^T+b2.
  - Only non-train (NT) rows of plp evolve; train (T) rows are constant after
    step 1.  We keep the state as table = dis * plp (dis = deg^-1/2), fp16,
    so per-edge messages need no norm multiply:
        plp_new[d] = dis[d] * ( sum_{e: src NT} table[src] + c )
    where c is a constant per dst: c1 (from dis*label_init over T srcs, used in
    step 1) or c2 (from dis*hard over T srcs, steps 2..10).
  - Nodes are permuted host-side: NT nodes first, padded per-core stripes.
    Each core owns a contiguous stripe of NT dst rows; edges are bucketed by
    (dst_tile of 128, src chunk of <=32768 rows) host-side, padded to uniform
    capacities across cores (SPMD), and gathered per iteration with
    gpsimd.dma_gather (256B elements) from an HBM fp16 table.
  - Scatter/segment-sum is done on the TensorEngine: per 128-edge slot a
    host-precomputed fp8 selector S (S[e,d]=1 iff dst_local(e)==d) multiplies
    the gathered messages, accumulating in PSUM per dst tile.
  - Per-iteration halo exchange: AllGather of each core's new compact fp16
    rows, then a strided DMA expands them into the 256B-strided table.
"""

import math
import os
import sys

import numpy as np

sys.path.insert(0, "/opt/trn_rl_repo")

import ml_dtypes  # noqa: E402

import concourse.bass as bass  # noqa: E402
import concourse.mybir as mybir  # noqa: E402
import concourse.tile as tile  # noqa: E402
from concourse import bacc  # noqa: E402
from concourse.bass_utils import run_bass_kernel_spmd  # noqa: E402

P = 128
NCORES = 8
TPAD = 128  # fp16 elements per table row (256B, dma_gather elem granularity)
GROUP = 7  # dst tiles per dma_gather call group
MAX_CALL = int(os.environ.get("KERNEL_MAX_CALL", "1024"))
# 2 SWDGE queues overlap gather desc-gen/transfer across calls (~31% total
# win measured); 4 queues hangs the device on this runtime, as does
# MAX_CALL>1024 or a larger descriptor ring.
NQUEUES = int(os.environ.get("KERNEL_QUEUES", "2"))

F16 = mybir.dt.float16
F32 = mybir.dt.float32
F8 = mybir.dt.float8e4
I16 = mybir.dt.int16
NP_F8 = ml_dtypes.float8_e4m3


def _ceil(a, b):
    return -(-a // b)


class BuildOnly(Exception):
    pass


class EdgePlan:
    """Host-side bucketed edge plan for one SpMM pass, uniform across cores.

    src_row: int array, row index into the pass's gather table
    dst_pid: int array, padded NT id of the destination
    """

    def __init__(self, src_row, dst_pid, n_rows, s_pad, n_tiles):
        self.n_chunks = max(1, _ceil(n_rows, 32768))
        self.chunk = _ceil(n_rows, self.n_chunks)
        self.n_tiles = n_tiles
        nch = self.n_chunks

        core = dst_pid // s_pad
        dloc = dst_pid - core * s_pad
        tl = dloc // P
        dstloc = dloc % P
        ch = src_row // self.chunk

        key = (core * n_tiles + tl) * nch + ch
        counts = np.bincount(key, minlength=NCORES * n_tiles * nch).reshape(
            NCORES, n_tiles, nch
        )
        caps = counts.max(axis=0)  # [n_tiles, nch]
        caps = ((caps + P - 1) // P) * P
        self.caps = caps
        self.slots_per_tile = caps.sum(axis=1) // P  # [n_tiles]
        self.s_off = np.concatenate([[0], np.cumsum(self.slots_per_tile)])
        self.total_slots = int(self.s_off[-1])

        # per (chunk, group) call: num idxs and per-tile column offsets
        self.n_groups = _ceil(n_tiles, GROUP)
        self.call_num = np.zeros((nch, self.n_groups), dtype=np.int64)
        self.buck_col = np.zeros((nch, n_tiles), dtype=np.int64)  # col in its call buf
        for c in range(nch):
            for g in range(self.n_groups):
                off = 0
                for t in range(g * GROUP, min((g + 1) * GROUP, n_tiles)):
                    self.buck_col[c, t] = off
                    off += caps[t, c] // P
                self.call_num[c, g] = off * P
        # col offset of each call inside the flat idx stream (per chunk then group)
        self.call_off = np.zeros((nch, self.n_groups), dtype=np.int64)
        off = 0
        for c in range(nch):
            for g in range(self.n_groups):
                self.call_off[c, g] = off
                off += self.call_num[c, g]
        self.total_idx = off

        # sub-calls of <= MAX_CALL idxs: per (c, g) a list of (idx_off, num, col0)
        self.subcalls = {}
        for c in range(nch):
            for g in range(self.n_groups):
                num = int(self.call_num[c, g])
                base = int(self.call_off[c, g])
                subs = []
                p0 = 0
                while p0 < num:
                    n_ = min(MAX_CALL, num - p0)
                    subs.append((base + p0, n_, p0 // P))
                    p0 += n_
                self.subcalls[(c, g)] = subs

        # order edges by (core, chunk, tile); build padded per-core streams
        order = np.argsort((core * nch + ch) * n_tiles + tl, kind="stable")
        src_o = src_row[order]
        core_o = core[order]
        ch_o = ch[order]
        tl_o = tl[order]
        dst_o = dstloc[order]

        # destination position of each edge in the padded stream
        # padded stream order: for chunk c, group g, tile t in g: cap[t,c] entries
        base_tc = np.zeros((nch, n_tiles), dtype=np.int64)
        for c in range(nch):
            for g in range(self.n_groups):
                for t in range(g * GROUP, min((g + 1) * GROUP, n_tiles)):
                    base_tc[c, t] = self.call_off[c, g] + self.buck_col[c, t] * P

        self.idx16 = np.zeros((NCORES, self.total_idx), dtype=np.int16)
        self.dstloc = np.full((NCORES, self.total_idx), -1, dtype=np.int16)
        # rank of each edge within its (core, chunk, tile) bucket
        grp_key = (core_o * nch + ch_o) * n_tiles + tl_o
        # stable sort keeps original order; compute rank via cumcount
        uniq, inv, cnt = np.unique(grp_key, return_inverse=True, return_counts=True)
        starts = np.concatenate([[0], np.cumsum(cnt)])[:-1]
        rank = np.arange(len(grp_key)) - starts[inv]
        pos = base_tc[ch_o, tl_o] + rank
        self.idx16[core_o, pos] = (src_o - ch_o * self.chunk).astype(np.int16)
        self.dstloc[core_o, pos] = dst_o.astype(np.int16)

    def wrapped_idx(self, core):
        """[128, total_idx//16] int16, wrapped-16 and replicated to 8 groups."""
        v = self.idx16[core].reshape(-1, 16).T  # [16, total/16]
        return np.tile(v, (8, 1)).copy()

    def s_blob(self, core):
        """[128, total_slots*128] fp8: per slot S[e,d] = (dstloc[e]==d).

        Slot order: tile-major (tile t: its chunk-0 slots then chunk-1 slots),
        matching the matmul loop.  Column range of tile t: s_off[t]*128.
        """
        nch = self.n_chunks
        out = np.zeros((P, self.total_slots * P), dtype=NP_F8)
        iota = np.arange(P, dtype=np.int16)
        for t in range(self.n_tiles):
            si = self.s_off[t]
            for c in range(nch):
                nsl = self.caps[t, c] // P
                if nsl == 0:
                    continue
                g = t // GROUP
                base = self.call_off[c, g] + self.buck_col[c, t] * P
                d = self.dstloc[core, base : base + nsl * P].reshape(nsl, P)
                # S [slot, e, d]
                s = (d[:, :, None] == iota[None, None, :]).astype(NP_F8)
                # [P(e), nsl, P(d)] -> columns
                out[:, si * P : (si + nsl) * P] = (
                    s.transpose(1, 0, 2).reshape(P, nsl * P)
                )
                si += nsl
        return out


def _build_program(pm, pc, n_t, s_pad, st_pad, tn, tt):
    """pm: main-pass EdgePlan (NT->NT), pc: c-pass plan (T->NT)."""
    nt_pad = NCORES * s_pad
    nc = bacc.Bacc(
        None,
        target_bir_lowering=False,
        num_devices=NCORES,
        num_swdge_queues=NQUEUES,
    )

    def param(name, shape, dt, out=False):
        return nc.declare_dram_parameter(name, list(shape), dt, isOutput=out)

    tbl_init = param("tbl_init", (nt_pad, TPAD), F16)
    tbl_t1 = param("tbl_t1", (pc.n_chunks * pc.chunk, TPAD), F16)
    tbl_t2 = param("tbl_t2", (pc.n_chunks * pc.chunk, TPAD), F16)
    idx_nt = param("idx_nt", (P, pm.total_idx // 16), I16)
    idx_t = param("idx_t", (P, pc.total_idx // 16), I16)
    s_nt = param("s_nt", (P, pm.total_slots * P), F8)
    s_t = param("s_t", (P, pc.total_slots * P), F8)
    xnt = param("xnt", (512, s_pad), F16)  # pre-transposed on host
    xt = param("xt", (512, st_pad), F16)
    w1t = param("w1t", (512, 256), F16)
    b1 = param("b1", (256, 1), F32)
    w2t = param("w2t", (256, 40), F16)
    b2b = param("b2b", (P, 40), F32)
    alpha_nt = param("alpha_nt", (s_pad, 1), F32)
    alpha_t = param("alpha_t", (st_pad, 1), F32)
    dis_nt = param("dis_nt", (s_pad, 1), F32)
    dissq_nt = param("dissq_nt", (s_pad, 1), F32)
    hard_t = param("hard_t", (st_pad, 40), F32)
    out_nt = param("out_nt", (s_pad, 40), F32, out=True)
    out_t = param("out_t", (st_pad, 40), F32, out=True)

    table = nc.dram_tensor("table", [nt_pad, TPAD], F16)
    # ping-pong the collective in/out buffers: a lagging peer may still be
    # pulling iteration k's data after our collective instruction completed,
    # so iteration k+1 must not overwrite the same buffers
    cown = [nc.dram_tensor(f"cown{i}", [s_pad, 40], F16) for i in range(2)]
    callg = [
        nc.dram_tensor(f"callg{i}", [nt_pad, 40], F16, addr_space="Shared")
        for i in range(2)
    ]

    RG = [list(range(NCORES))]

    with tile.TileContext(nc) as tc:
        with (
            tc.tile_pool(name="persist", bufs=1) as pp,
            tc.tile_pool(name="work", bufs=4) as wp,
            tc.tile_pool(name="gpool", bufs=4) as gp,
            tc.tile_pool(name="spool", bufs=3) as sp,
            tc.tile_pool(name="mpsum", bufs=2, space="PSUM") as mp,
            tc.tile_pool(name="apsum", bufs=4, space="PSUM") as ap_,
        ):
            # one-time init: fills pad columns so later strided updates leave
            # only finite data for gathers
            nc.sync.dma_start(out=table[:, :], in_=tbl_init[:, :])

            # ---- persistent SBUF ----
            idxm_sb = pp.tile([P, pm.total_idx // 16], I16, tag="idxm")
            nc.sync.dma_start(out=idxm_sb[:], in_=idx_nt[:, :])
            idxc_sb = pp.tile([P, pc.total_idx // 16], I16, tag="idxc")
            nc.sync.dma_start(out=idxc_sb[:], in_=idx_t[:, :])

            ft_nt = pp.tile([P, tn, 40], F32, tag="ftnt")
            ft_t = pp.tile([P, tt, 40], F32, tag="ftt")
            c1 = pp.tile([P, tn, 40], F32, tag="c1")
            c2 = pp.tile([P, tn, 40], F32, tag="c2")
            compact = pp.tile([P, tn, 40], F16, tag="compact")

            w1_sb = pp.tile([P, 4, 256], F16, tag="w1")
            nc.sync.dma_start(
                out=w1_sb[:], in_=w1t.ap().rearrange("(k p) h -> p k h", p=P)
            )
            w2_sb = pp.tile([P, 2, 40], F16, tag="w2")
            nc.sync.dma_start(
                out=w2_sb[:], in_=w2t.ap().rearrange("(h p) c -> p h c", p=P)
            )
            b1_sb = pp.tile([P, 2], F32, tag="b1")
            nc.sync.dma_start(
                out=b1_sb[:], in_=b1.ap().rearrange("(h p) o -> p (h o)", p=P)
            )
            b2_sb = pp.tile([P, 40], F32, tag="b2")
            nc.sync.dma_start(out=b2_sb[:], in_=b2b[:, :])

            def cols_load(prm, n_tiles, tag):
                t_ = pp.tile([P, n_tiles], F32, tag=tag)
                nc.sync.dma_start(
                    out=t_[:], in_=prm.ap().rearrange("(t p) o -> p (t o)", p=P)
                )
                return t_

            disn_sb = cols_load(dis_nt, tn, "disn")
            dsqn_sb = cols_load(dissq_nt, tn, "dsqn")
            aln_sb = cols_load(alpha_nt, tn, "aln")
            alt_sb = cols_load(alpha_t, tt, "alt")

            # sigmoid(alpha); a*dis; 1-a
            sign_sb = pp.tile([P, tn], F32, tag="sign")
            nc.scalar.activation(
                sign_sb[:], aln_sb[:], mybir.ActivationFunctionType.Sigmoid
            )
            sigt_sb = pp.tile([P, tt], F32, tag="sigt")
            nc.scalar.activation(
                sigt_sb[:], alt_sb[:], mybir.ActivationFunctionType.Sigmoid
            )
            disa_sb = pp.tile([P, tn], F32, tag="disa")
            nc.vector.tensor_tensor(
                out=disa_sb[:], in0=sign_sb[:], in1=disn_sb[:],
                op=mybir.AluOpType.mult,
            )
            oman_sb = pp.tile([P, tn], F32, tag="oman")
            nc.vector.tensor_scalar(
                out=oman_sb[:], in0=sign_sb[:], scalar1=-1.0, scalar2=1.0,
                op0=mybir.AluOpType.mult, op1=mybir.AluOpType.add,
            )
            omat_sb = pp.tile([P, tt], F32, tag="omat")
            nc.vector.tensor_scalar(
                out=omat_sb[:], in0=sigt_sb[:], scalar1=-1.0, scalar2=1.0,
                op0=mybir.AluOpType.mult, op1=mybir.AluOpType.add,
            )

            # ---- MLP (FT branch) ----
            def mlp(xsrc, n_tiles, ft_dst):
                for n in range(n_tiles):
                    xTs = []
                    for k in range(4):
                        xT = wp.tile([P, P], F16, tag="xT")
                        nc.sync.dma_start(
                            out=xT[:],
                            in_=xsrc[k * P : (k + 1) * P, n * P : (n + 1) * P],
                        )
                        xTs.append(xT)
                    ps2 = mp.tile([P, 40], F32, tag="ps2")
                    for h in range(2):
                        ps1 = mp.tile([P, P], F32, tag="ps1")
                        for k in range(4):
                            nc.tensor.matmul(
                                ps1[:],
                                lhsT=w1_sb[:, k, h * P : (h + 1) * P],
                                rhs=xTs[k][:],
                                start=(k == 0),
                                stop=(k == 3),
                            )
                        hT = wp.tile([P, P], F16, tag="hT")
                        nc.scalar.activation(
                            hT[:], ps1[:], mybir.ActivationFunctionType.Relu,
                            bias=b1_sb[:, h : h + 1],
                        )
                        nc.tensor.matmul(
                            ps2[:], lhsT=hT[:], rhs=w2_sb[:, h, :],
                            start=(h == 0), stop=(h == 1),
                        )
                    nc.vector.tensor_tensor(
                        out=ft_dst[:, n, :], in0=ps2[:], in1=b2_sb[:],
                        op=mybir.AluOpType.add,
                    )

            mlp(xnt, tn, ft_nt)
            mlp(xt, tt, ft_t)

            # ---- generic SpMM pass ----
            _regs = {}

            def num_reg(v):
                if v not in _regs:
                    _regs[v] = nc.gpsimd.to_reg(v)
                return _regs[v]

            def spmm_pass(plan, tsrc, idx_sb, s_param, evac):
                """tsrc: DRAM table. evac(t, psum_ap) -> emits eviction."""
                nch = plan.n_chunks
                for g in range(plan.n_groups):
                    gbufs = []
                    for c in range(nch):
                        num = int(plan.call_num[c, g])
                        if num == 0:
                            gbufs.append(None)
                            continue
                        gb = gp.tile([P, num // P, TPAD], F16, tag="gb")
                        r0 = c * plan.chunk
                        nrow = plan.chunk
                        if os.environ.get("KERNEL_NO_GATHER", "0") == "1":
                            # debug: sequential read instead of gather
                            nc.sync.dma_start(
                                out=gb[:],
                                in_=tsrc[r0 : r0 + num, :].rearrange(
                                    "(n p) e -> p n e", p=P
                                ),
                            )
                        else:
                            for off, n_, col0 in plan.subcalls[(c, g)]:
                                nc.gpsimd.dma_gather(
                                    out_ap=gb[:, col0 : col0 + n_ // P, :],
                                    in_ap=tsrc[r0 : r0 + nrow, :],
                                    idxs_ap=idx_sb[:, off // 16 : (off + n_) // 16],
                                    num_idxs=n_,
                                    num_idxs_reg=num_reg(n_),
                                    elem_size=TPAD,
                                    queue_num=spmm_pass.qi % NQUEUES,
                                )
                                spmm_pass.qi += 1
                        gbufs.append(gb)
                    for t in range(g * GROUP, min((g + 1) * GROUP, plan.n_tiles)):
                        tot = int(plan.slots_per_tile[t])
                        if tot == 0:
                            continue
                        si = int(plan.s_off[t])
                        st_ = sp.tile([P, tot * P], F8, tag="sstr")
                        nc.sync.dma_start(
                            out=st_[:], in_=s_param[:, si * P : (si + tot) * P]
                        )
                        ps = ap_.tile([P, 40], F32, tag="acc")
                        k = 0
                        for c in range(nch):
                            nsl = int(plan.caps[t, c]) // P
                            bc = int(plan.buck_col[c, t])
                            for j in range(nsl):
                                nc.tensor.matmul(
                                    ps[:],
                                    lhsT=st_[:, k * P : (k + 1) * P],
                                    rhs=gbufs[c][:, bc + j, 0:40],
                                    start=(k == 0),
                                    stop=(k == tot - 1),
                                )
                                k += 1
                        evac(t, ps)

            # ---- c1 / c2 passes (T sources; streamed fp8 S) ----
            def evac_c(dst):
                def f(t, ps):
                    nc.vector.tensor_copy(out=dst[:, t, :], in_=ps[:])
                return f

            spmm_pass.qi = 0
            spmm_pass(pc, tbl_t1, idxc_sb, s_t, evac_c(c1))
            spmm_pass(pc, tbl_t2, idxc_sb, s_t, evac_c(c2))

            # ---- 10 PLP iterations ----
            for it in range(10):
                tsrc = tbl_init if it == 0 else table
                cbuf = c1 if it == 0 else c2

                if it < 9:
                    def evac_iter(t, ps, cbuf=cbuf):
                        tmp = wp.tile([P, 40], F32, tag="ev")
                        nc.vector.tensor_tensor(
                            out=tmp[:], in0=ps[:], in1=cbuf[:, t, :],
                            op=mybir.AluOpType.add,
                        )
                        nc.vector.tensor_scalar(
                            out=compact[:, t, :], in0=tmp[:],
                            scalar1=dsqn_sb[:, t : t + 1], scalar2=None,
                            op0=mybir.AluOpType.mult,
                        )
                else:
                    def evac_iter(t, ps, cbuf=cbuf):
                        tmp = wp.tile([P, 40], F32, tag="ev")
                        nc.vector.tensor_tensor(
                            out=tmp[:], in0=ps[:], in1=cbuf[:, t, :],
                            op=mybir.AluOpType.add,
                        )
                        t2 = wp.tile([P, 40], F32, tag="ev2")
                        nc.vector.tensor_scalar(
                            out=t2[:], in0=tmp[:],
                            scalar1=disa_sb[:, t : t + 1], scalar2=None,
                            op0=mybir.AluOpType.mult,
                        )
                        t3 = wp.tile([P, 40], F32, tag="ev3")
                        nc.vector.tensor_scalar(
                            out=t3[:], in0=ft_nt[:, t, :],
                            scalar1=oman_sb[:, t : t + 1], scalar2=None,
                            op0=mybir.AluOpType.mult,
                        )
                        t4 = wp.tile([P, 40], F32, tag="ev4")
                        nc.vector.tensor_tensor(
                            out=t4[:], in0=t2[:], in1=t3[:],
                            op=mybir.AluOpType.add,
                        )
                        nc.sync.dma_start(
                            out=out_nt[t * P : (t + 1) * P, :], in_=t4[:]
                        )

                spmm_pass(pm, tsrc, idxm_sb, s_nt, evac_iter)

                if it < 9:
                    cw, cg = cown[it % 2], callg[it % 2]
                    nc.sync.dma_start(
                        out=cw.ap().rearrange("(t p) c -> p t c", p=P),
                        in_=compact[:],
                    )
                    if os.environ.get("KERNEL_NO_CC", "0") == "1":
                        # debug mode: skip the collective (wrong cross-core data)
                        nc.sync.dma_start(
                            out=cg[0 : s_pad, :], in_=cw[:, :]
                        )
                    else:
                        nc.gpsimd.collective_compute(
                            "AllGather",
                            mybir.AluOpType.bypass,
                            replica_groups=RG,
                            ins=[cw.ap().opt()],
                            outs=[cg.ap().opt()],
                        )
                    half = nt_pad // 2
                    nc.sync.dma_start(
                        out=table[0:half, 0:40], in_=cg[0:half, :]
                    )
                    nc.sync.dma_start(
                        out=table[half:nt_pad, 0:40], in_=cg[half:nt_pad, :]
                    )

            # ---- T-side final combine ----
            for t in range(tt):
                hsb = wp.tile([P, 40], F32, tag="hsb")
                nc.sync.dma_start(out=hsb[:], in_=hard_t[t * P : (t + 1) * P, :])
                t1_ = wp.tile([P, 40], F32, tag="tc1")
                nc.vector.tensor_scalar(
                    out=t1_[:], in0=hsb[:], scalar1=sigt_sb[:, t : t + 1],
                    scalar2=None, op0=mybir.AluOpType.mult,
                )
                t2_ = wp.tile([P, 40], F32, tag="tc2")
                nc.vector.tensor_scalar(
                    out=t2_[:], in0=ft_t[:, t, :], scalar1=omat_sb[:, t : t + 1],
                    scalar2=None, op0=mybir.AluOpType.mult,
                )
                t3_ = wp.tile([P, 40], F32, tag="tc3")
                nc.vector.tensor_tensor(
                    out=t3_[:], in0=t1_[:], in1=t2_[:],
                    op=mybir.AluOpType.add,
                )
                nc.sync.dma_start(out=out_t[t * P : (t + 1) * P, :], in_=t3_[:])

    nc.compile()
    return nc


def kernel(**inputs):
    x = np.asarray(inputs["x"], dtype=np.float32)
    edge_index = np.asarray(inputs["edge_index"])
    label_init = np.asarray(inputs["label_init"], dtype=np.float32)
    train_mask = np.asarray(inputs["train_mask"]).astype(bool)
    hard = np.asarray(inputs["hard_one_hot"], dtype=np.float32)
    fc1_w = np.asarray(inputs["fc1_w"], dtype=np.float32)
    fc1_b = np.asarray(inputs["fc1_b"], dtype=np.float32)
    fc2_w = np.asarray(inputs["fc2_w"], dtype=np.float32)
    fc2_b = np.asarray(inputs["fc2_b"], dtype=np.float32)
    alpha = np.asarray(inputs["alpha"], dtype=np.float32)

    n = x.shape[0]
    row = edge_index[0].astype(np.int64)
    col = edge_index[1].astype(np.int64)

    deg = np.bincount(row, minlength=n).astype(np.float64) + 1.0
    dis = (1.0 / np.sqrt(deg)).astype(np.float32)

    nt_ids = np.nonzero(~train_mask)[0]
    t_ids = np.nonzero(train_mask)[0]
    n_nt, n_t = len(nt_ids), len(t_ids)

    s_real = _ceil(n_nt, NCORES)
    tn = _ceil(s_real, P)
    s_pad = tn * P
    nt_pad = NCORES * s_pad
    st_real = _ceil(n_t, NCORES)
    tt = _ceil(st_real, P)
    st_pad = tt * P

    # padded NT id / compact T id for each original node
    pid = np.full(n, -1, dtype=np.int64)
    j = np.arange(n_nt)
    stripe = j // s_real
    pid[nt_ids] = stripe * s_pad + (j - stripe * s_real)
    tix = np.full(n, -1, dtype=np.int64)
    tix[t_ids] = np.arange(n_t)

    # edges into NT dsts
    sel = ~train_mask[col]
    es, ed = row[sel], col[sel]
    src_nt = ~train_mask[es]
    # main: NT->NT plus self-loops on NT
    m_src = np.concatenate([pid[es[src_nt]], pid[nt_ids]])
    m_dst = np.concatenate([pid[ed[src_nt]], pid[nt_ids]])
    pm = EdgePlan(m_src, m_dst, nt_pad, s_pad, tn)
    # cpass: T->NT
    c_src = tix[es[~src_nt]]
    c_dst = pid[ed[~src_nt]]
    pc = EdgePlan(c_src, c_dst, n_t, s_pad, tn)

    # ---- tables ----
    scaled_li = dis[:, None] * label_init  # [n, 40]
    scaled_hd = dis[:, None] * hard

    def pack_rows(rows40):
        out = np.zeros((rows40.shape[0], TPAD), dtype=np.float16)
        out[:, :40] = rows40.astype(np.float16)
        return out

    tbl_init_g = np.zeros((nt_pad, TPAD), dtype=np.float16)
    tbl_init_g[pid[nt_ids], :40] = scaled_li[nt_ids].astype(np.float16)
    t_rows = pc.n_chunks * pc.chunk
    tbl_t1_g = np.zeros((t_rows, TPAD), dtype=np.float16)
    tbl_t1_g[: n_t, :40] = scaled_li[t_ids].astype(np.float16)
    tbl_t2_g = np.zeros((t_rows, TPAD), dtype=np.float16)
    tbl_t2_g[: n_t, :40] = scaled_hd[t_ids].astype(np.float16)

    # ---- per-core MLP / combine inputs ----
    def stripe_rows(ids, srl, spad_, nstripes=NCORES):
        """Return [nstripes, spad_] original-id per padded slot (-1 pad)."""
        m = np.full((nstripes, spad_), -1, dtype=np.int64)
        for i in range(nstripes):
            lo = i * srl
            hi = min(len(ids), (i + 1) * srl)
            if hi > lo:
                m[i, : hi - lo] = ids[lo:hi]
        return m

    nt_map = stripe_rows(nt_ids, s_real, s_pad)
    t_map = stripe_rows(t_ids, st_real, st_pad)

    def take(arr, idmap, fill=0.0):
        out = np.full((idmap.shape[0], idmap.shape[1]) + arr.shape[1:], fill,
                      dtype=arr.dtype)
        valid = idmap >= 0
        out[valid] = arr[idmap[valid]]
        return out

    xnt_g = np.ascontiguousarray(
        take(x, nt_map).astype(np.float16).transpose(0, 2, 1)
    )
    xt_g = np.ascontiguousarray(take(x, t_map).astype(np.float16).transpose(0, 2, 1))
    al_nt_g = take(alpha, nt_map).astype(np.float32)
    al_t_g = take(alpha, t_map).astype(np.float32)
    dis_nt_g = take(dis[:, None], nt_map).astype(np.float32)
    dsq_nt_g = take((dis * dis)[:, None], nt_map).astype(np.float32)
    hard_t_g = take(hard, t_map).astype(np.float32)

    w1t_g = fc1_w.T.astype(np.float16).copy()  # [512, 256]
    b1_g = fc1_b.reshape(256, 1).astype(np.float32)
    w2t_g = fc2_w.T.astype(np.float16).copy()  # [256, 40]
    b2b_g = np.tile(fc2_b.reshape(1, 40), (P, 1)).astype(np.float32)

    nc = _build_program(pm, pc, n_t, s_pad, st_pad, tn, tt)

    if os.environ.get("KERNEL_BUILD_ONLY", "0") == "1":
        e = BuildOnly()
        e.nc = nc
        raise e

    in_maps = []
    for i in range(NCORES):
        in_maps.append(
            dict(
                tbl_init=tbl_init_g,
                tbl_t1=tbl_t1_g,
                tbl_t2=tbl_t2_g,
                idx_nt=pm.wrapped_idx(i),
                idx_t=pc.wrapped_idx(i),
                s_nt=pm.s_blob(i),
                s_t=pc.s_blob(i),
                xnt=xnt_g[i],
                xt=xt_g[i],
                w1t=w1t_g,
                b1=b1_g,
                w2t=w2t_g,
                b2b=b2b_g,
                alpha_nt=al_nt_g[i],
                alpha_t=al_t_g[i],
                dis_nt=dis_nt_g[i],
                dissq_nt=dsq_nt_g[i],
                hard_t=hard_t_g[i],
            )
        )

    if os.environ.get("KERNEL_SIM", "0") == "1":
        from concourse import bass_interp

        sim = bass_interp.MultiCoreSim(nc, NCORES)
        for i in range(NCORES):
            for k, v in in_maps[i].items():
                sim.cores[i].tensor(k)[:] = v
        sim.simulate()
        results = [
            {k: np.array(sim.cores[i].mem_tensor(k)) for k in ("out_nt", "out_t")}
            for i in range(NCORES)
        ]
        res = None
    else:
        res = run_bass_kernel_spmd(
            nc, in_maps, core_ids=list(range(NCORES)),
            trace=bool(int(os.environ.get("KERNEL_TRACE", "0"))),
        )
        results = res.results
        nbench = int(os.environ.get("KERNEL_BENCH", "0"))
        if nbench > 0:
            import time as _time

            times = []
            for _ in range(nbench):
                t0 = _time.time()
                run_bass_kernel_spmd(nc, in_maps, core_ids=list(range(NCORES)))
                times.append(_time.time() - t0)
            kernel.last_bench_s = min(times)
    kernel.last_results = res
    kernel.last_nc = nc
    kernel.last_in_maps = in_maps

    out = np.zeros((n, 40), dtype=np.float32)
    for i in range(NCORES):
        om = results[i]["out_nt"]
        ot = results[i]["out_t"]
        v = nt_map[i] >= 0
        out[nt_map[i][v]] = om[v]
        v = t_map[i] >= 0
        out[t_map[i][v]] = ot[v]
    return out

